# revision 1
# baseline (speedup 1.0000x reference)
"""Trainium2 Bass kernel for nn_DualAddressingPhasor.

Math: the phasor cumsum-bind/retrieve is causal linear attention:
  retrieved[l] = sum_{l'<=l} (sum_k cos(phi_l,k - phi_l',k)) * value[l']
Per 512-row chunk this is (1) a carried state [2K, D] = CS^T @ value over
the prefix plus (2) intra-chunk attention triu(Cc@Cc^T + Sc@Sc^T) @ value_c.

Sharding: 8 cores = 2 batches x 4 sequence chunks of 512. Uniform SPMD
program; per-core variation is entirely in the data (right-aligned
zero-padded prefix, host-precomputed positional sin/cos with zeros in the
padding so padded rows contribute nothing).
"""

import sys

for _p in ("/opt/trn_rl_repo",):
    if _p not in sys.path:
        sys.path.append(_p)

import numpy as np
import ml_dtypes

import concourse.bacc as bacc
import concourse.tile as tile
import concourse.mybir as mybir
from concourse.bass import ts
from concourse.bass_utils import run_bass_kernel_spmd
from concourse.masks import make_identity

F32 = mybir.dt.float32
F32R = mybir.dt.float32r
BF16 = mybir.dt.bfloat16
USE_BF16 = False
AF = mybir.ActivationFunctionType
ALU = mybir.AluOpType

D = 512
K = 32
B = 2
L = 2048
CH = 512          # chunk rows per core
T = 2048          # padded rows processed per core
NCORE = 8
HALF_PI = float(np.pi / 2)

_NC_CACHE = {}
LAST_RESULT = None
RUN_KWARGS = {}


def _build(zero_bv: bool):
    XDT = BF16 if USE_BF16 else F32R
    nc = bacc.Bacc("TRN2", num_devices=NCORE)

    xt = nc.dram_tensor("xt", [4, 128, 4, CH], XDT, kind="ExternalInput")
    w1f = nc.dram_tensor("w1f", [128, 4, 128], XDT, kind="ExternalInput")
    w1 = nc.dram_tensor("w1", [128, 4, D], XDT, kind="ExternalInput")
    w2 = nc.dram_tensor("w2", [128, 4, K], XDT, kind="ExternalInput")
    wv = nc.dram_tensor("wv", [128, 4, D], XDT, kind="ExternalInput")
    wo = nc.dram_tensor("wo", [128, 4, D], XDT, kind="ExternalInput")
    ncs = nc.dram_tensor("ncs", [1, D], XDT, kind="ExternalInput")
    bvr = nc.dram_tensor("bvr", [1, D], XDT, kind="ExternalInput")
    b1p = nc.dram_tensor("b1p", [128, 4], F32, kind="ExternalInput")
    b2p = nc.dram_tensor("b2p", [128, 1], F32, kind="ExternalInput")
    pcos = nc.dram_tensor("pcos", [128, CH], F32, kind="ExternalInput")
    psin = nc.dram_tensor("psin", [128, CH], F32, kind="ExternalInput")
    sgc = nc.dram_tensor("sgc", [128, CH], F32, kind="ExternalInput")
    sgs = nc.dram_tensor("sgs", [128, CH], F32, kind="ExternalInput")
    res = nc.dram_tensor("res", [128, 4, D], F32, kind="ExternalInput")
    xn = nc.dram_tensor("xn", [128, 12, D], XDT, kind="ExternalInput")
    y = nc.dram_tensor("y", [CH, D], F32, kind="ExternalOutput")

    kc = nc.dram_tensor("kc", [128, 1], F32, kind="ExternalInput")  # pi*content_scale

    with tile.TileContext(nc) as tc:
        with (
            tc.tile_pool(name="const", bufs=1) as cp_,
            tc.tile_pool(name="big", bufs=1) as bigp,
            tc.tile_pool(name="rot", bufs=2) as rot,
            tc.tile_pool(name="rot4", bufs=4) as rot4,
            tc.tile_pool(name="pmm", bufs=4, space="PSUM") as pmm,
            tc.tile_pool(name="pone", bufs=1, space="PSUM") as pone,
            tc.tile_pool(name="ptr", bufs=2, space="PSUM") as ptrp,
        ):
            # ---- input loads, ordered so the first (a) matmuls start ASAP ----
            xt_sb = bigp.tile([128, 4, T], XDT)
            w1_sb = cp_.tile([128, 4, D], XDT)
            w2_sb = cp_.tile([128, 4, K], XDT)
            wv_sb = cp_.tile([128, 4, D], XDT)
            wo_sb = cp_.tile([128, 4, D], XDT)
            res_sb = cp_.tile([128, 4, D], F32)
            bvr_sb = cp_.tile([1, D], XDT)
            b1p_sb = cp_.tile([128, 4], F32)
            b2p_sb = cp_.tile([128, 1], F32)
            kc_sb = cp_.tile([128, 1], F32)
            pcos_sb = cp_.tile([128, CH], F32)
            psin_sb = cp_.tile([128, CH], F32)
            ncs_sb = cp_.tile([1, D], XDT)

            # early-need: xt on sync, w1/w2/wv on gpsimd, small aux on scalar.
            # late-need (xn, wo, res) go behind xt on the sync queue so they
            # don't steal HBM bandwidth from the (a) matmuls.
            # first-needed pair: dout0 slice of w1 + chunk0 of xt (2 descriptors)
            nc.sync.dma_start(w1_sb[:, :, 0:128], w1f[:])
            nc.gpsimd.dma_start(xt_sb[:, :, ts(0, CH)], xt[0])
            nc.sync.dma_start(w1_sb[:, :, 128:512], w1[:, :, 128:512])
            nc.gpsimd.dma_start(xt_sb[:, :, ts(1, CH)], xt[1])
            nc.sync.dma_start(xt_sb[:, :, ts(2, CH)], xt[2])
            nc.sync.dma_start(xt_sb[:, :, ts(3, CH)], xt[3])
            nc.gpsimd.dma_start(w2_sb[:], w2[:])
            nc.gpsimd.dma_start(wv_sb[:], wv[:])
            nc.scalar.dma_start(b1p_sb[:], b1p[:])
            nc.scalar.dma_start(b2p_sb[:], b2p[:])
            nc.scalar.dma_start(kc_sb[:], kc[:])
            nc.scalar.dma_start(pcos_sb[:], pcos[:])
            nc.scalar.dma_start(psin_sb[:], psin[:])
            sgc_sb = cp_.tile([128, CH], F32)
            nc.scalar.dma_start(sgc_sb[:], sgc[:])
            sgs_sb = cp_.tile([128, CH], F32)
            nc.scalar.dma_start(sgs_sb[:], sgs[:])
            nc.scalar.dma_start(bvr_sb[:], bvr[:])
            xn_sb = bigp.tile([128, 12, D], XDT)
            nc.sync.dma_start(xn_sb[:], xn[:])
            nc.gpsimd.dma_start(wo_sb[:], wo[:])
            nc.sync.dma_start(res_sb[:], res[:])
            nc.scalar.dma_start(ncs_sb[:], ncs[:])

            onesf = cp_.tile([128, 128], F32)
            nc.vector.memset(onesf[:], 1.0)
            onesr = cp_.tile([1, 128], XDT)
            nc.vector.tensor_copy(onesr[:], onesf[0:1, :])
            onesc = cp_.tile([128, 1], XDT)
            nc.vector.tensor_copy(onesc[:], onesf[:, 0:1])
            onescol_x = onesc
            halfpi = cp_.tile([128, 1], F32)
            nc.vector.memset(halfpi[:], HALF_PI)
            epsb = cp_.tile([128, 1], F32)
            nc.vector.memset(epsb[:], 1e-5)

            identf = cp_.tile([128, 128], F32)
            make_identity(nc, identf[:])
            identr = cp_.tile([128, 128], F32R)
            nc.vector.tensor_copy(identr[:], identf[:])

            # triangular masks for intra-chunk causal attention (lhsT form:
            # tri[p, tr, y] = 1 iff y >= p + 128*tr)
            tri = cp_.tile([128, 4, CH], F32)
            for tr in range(4):
                nc.gpsimd.memset(tri[:, tr, :], 0.0)
                nc.gpsimd.affine_select(
                    out=tri[:, tr, :], in_=tri[:, tr, :],
                    compare_op=ALU.is_gt, fill=1.0, base=128 * tr,
                    pattern=[[-1, CH]], channel_multiplier=1,
                )

            # ---- (a) h^T = tanh(W1^T x^T + b1), chunked; (b) packed content phase ----
            tt_sb = cp_.tile([128, CH], F32)
            for c in range(4):
                h_ck = rot.tile([128, 4, CH], XDT, tag="hck")
                for dout in range(4):
                    ph = pmm.tile([128, CH], F32, tag="pmm")
                    for k in range(4):
                        nc.tensor.matmul(
                            ph[:], w1_sb[:, k, ts(dout, 128)],
                            xt_sb[:, k, ts(c, CH)],
                            start=(k == 0), stop=(k == 3),
                        )
                    nc.scalar.activation(
                        h_ck[:, dout, :], ph[:], AF.Tanh,
                        bias=b1p_sb[:, dout : dout + 1], scale=1.0,
                    )
                pc = pmm.tile([32, CH], F32, tag="pmm")
                for k in range(4):
                    nc.tensor.matmul(
                        pc[:], w2_sb[:, k, :], h_ck[:, k, :],
                        start=(k == 0), stop=(k == 3),
                    )
                nc.scalar.activation(
                    tt_sb[32 * c : 32 * c + 32, :], pc[:], AF.Tanh,
                    bias=b2p_sb[0:32, :], scale=1.0,
                )

            # ---- phases: S = sgn_s*sin(ps + kc*t), C = sgn_c*sin(pc2 + kc*t)
            # (host folds pos phase to [-pi/2, pi/2] plus a sign; |kc*t|<=pi/2
            #  keeps the Sin LUT inside its accurate [-pi, pi] window)
            phS = cp_.tile([128, CH], F32)
            nc.vector.scalar_tensor_tensor(
                out=phS[:], in0=tt_sb[:], scalar=kc_sb[:], in1=psin_sb[:],
                op0=ALU.mult, op1=ALU.add,
            )
            s0 = cp_.tile([128, CH], F32)
            nc.scalar.activation(s0[:], phS[:], AF.Sin)
            spk = cp_.tile([128, CH], F32R)
            nc.vector.tensor_mul(spk[:], s0[:], sgs_sb[:])
            phC = cp_.tile([128, CH], F32)
            nc.vector.scalar_tensor_tensor(
                out=phC[:], in0=tt_sb[:], scalar=kc_sb[:], in1=pcos_sb[:],
                op0=ALU.mult, op1=ALU.add,
            )
            c0 = cp_.tile([128, CH], F32)
            nc.scalar.activation(c0[:], phC[:], AF.Sin)
            cpk = cp_.tile([128, CH], F32R)
            nc.vector.tensor_mul(cpk[:], c0[:], sgc_sb[:])

            # combined chunk CS tile [64, CH] at base 0 (C rows 0:32, S 32:64)
            csc = cp_.tile([64, CH], F32R)
            nc.vector.tensor_copy(csc[0:32, :], cpk[96:128, :])
            nc.vector.tensor_copy(csc[32:64, :], spk[96:128, :])

            # ---- (c) value = x @ Wv + bv for the own chunk only ----
            value_sb = bigp.tile([128, 4, D], F32R)
            for tt in range(4):
                pv = pmm.tile([128, D], F32, tag="pmm")
                for k in range(4):
                    nc.tensor.matmul(
                        pv[:], xt_sb[:, k, ts(12 + tt, 128)], wv_sb[:, k, :],
                        start=(k == 0), stop=(zero_bv and k == 3),
                    )
                if not zero_bv:
                    nc.tensor.matmul(pv[:], onesr[:], bvr_sb[:], start=False, stop=True)
                if tt % 2 == 0:
                    nc.vector.tensor_copy(value_sb[:, tt, :], pv[:])
                else:
                    nc.scalar.copy(value_sb[:, tt, :], pv[:])

            # ---- transpose C/S to natural layout for the state matmul ----
            # csm[p, c, b, j]: natural row l = 512c + 128b + p, freq j (0:32 C, 32:64 S)
            csm = cp_.tile([128, 4, 4, 64], XDT)
            for src, j0 in ((cpk, 0), (spk, 32)):
                for bb in range(4):
                    ptr = ptrp.tile([128, 128], F32R, tag="ptr")
                    nc.tensor.transpose(ptr[:], src[:, ts(bb, 128)], identr[:])
                    nc.vector.tensor_copy(
                        csm[:, :, bb, j0 : j0 + 32],
                        ptr[:].rearrange("p (c j) -> p c j", j=32),
                    )

            # ---- (d) prefix state = (CSm^T X) @ Wv  (+ msum*bv) ----
            pg = pone.tile([64, D], F32, tag="pst")
            for kt in range(12):
                c, bb = kt // 4, kt % 4
                nc.tensor.matmul(
                    pg[:], csm[:, c, bb, :], xn_sb[:, kt, :],
                    start=(kt == 0), stop=(kt == 11),
                )
            g_sb = cp_.tile([64, D], F32R)
            nc.vector.tensor_copy(g_sb[:], pg[:])
            gt_sb = cp_.tile([128, 4, 64], XDT)
            for kk in range(4):
                ptg = ptrp.tile([128, 128], F32R, tag="ptr")
                nc.tensor.transpose(
                    ptg[0:128, 0:64], g_sb[:, ts(kk, 128)], identr[0:64, 0:64]
                )
                nc.vector.tensor_copy(gt_sb[:, kk, :], ptg[:, 0:64])
            pst = pone.tile([64, D], F32, tag="pst")
            for kk in range(4):
                nc.tensor.matmul(
                    pst[:], gt_sb[:, kk, :], wv_sb[:, kk, :],
                    start=(kk == 0), stop=(zero_bv and kk == 3),
                )
            if not zero_bv:
                # msum[j] = sum_l CSm[l, j]; state += msum (x) bv
                pms = ptrp.tile([64, 1], F32, tag="ptr")
                for kt in range(12):
                    c, bb = kt // 4, kt % 4
                    nc.tensor.matmul(
                        pms[:], csm[:, c, bb, :], onescol_x[:],
                        start=(kt == 0), stop=(kt == 11),
                    )
                ms_sb = cp_.tile([64, 1], XDT)
                nc.vector.tensor_copy(ms_sb[:], pms[:])
                msT = cp_.tile([1, 64], XDT)
                ptm = ptrp.tile([128, 128], F32R, tag="ptr")
                nc.tensor.transpose(ptm[0:1, 0:64], ms_sb[:], identr[0:64, 0:64])
                nc.vector.tensor_copy(msT[:], ptm[0:1, 0:64])
                nc.tensor.matmul(pst[:], msT[:], bvr_sb[:], start=False, stop=True)
            state_sb = cp_.tile([64, D], F32R)
            nc.vector.tensor_copy(state_sb[:], pst[:])

            # ---- (e) intra-chunk scores, triu-masked ----
            p_sb = cp_.tile([128, 4, CH], F32R)
            for tr in range(4):
                psc = pmm.tile([128, CH], F32, tag="pmm")
                nc.tensor.matmul(
                    psc[:], csc[:, ts(tr, 128)], csc[:],
                    start=True, stop=True,
                )
                nc.vector.tensor_mul(p_sb[:, tr, :], psc[:], tri[:, tr, :])

            # ---- (f)+(g) retrieved^T [D, CH] ----
            retrT = cp_.tile([128, 4, CH], XDT)
            sq_sb = cp_.tile([128, 4, CH], XDT)
            for dd in range(4):
                pr = pmm.tile([128, CH], F32, tag="pmm")
                for tr in range(4):
                    nc.tensor.matmul(
                        pr[:], value_sb[:, tr, ts(dd, 128)], p_sb[:, tr, :],
                        start=(tr == 0), stop=False,
                    )
                nc.tensor.matmul(
                    pr[:], state_sb[:, ts(dd, 128)], csc[:],
                    start=False, stop=True,
                )
                nc.vector.tensor_copy(retrT[:, dd, :], pr[:])
                nc.scalar.square(sq_sb[:, dd, :], pr[:])

            # ---- LayerNorm stats (feature dim = partitions here) ----
            ps_stat = pone.tile([1, 2 * CH], F32, tag="pst")
            for dd in range(4):
                nc.tensor.matmul(
                    ps_stat[0:1, 0:CH], onesc[:], retrT[:, dd, :],
                    start=(dd == 0), stop=(dd == 3),
                )
            for dd in range(4):
                nc.tensor.matmul(
                    ps_stat[0:1, CH : 2 * CH], onesc[:], sq_sb[:, dd, :],
                    start=(dd == 0), stop=(dd == 3),
                )
            mu_n = cp_.tile([1, CH], XDT)
            nc.vector.tensor_scalar_mul(mu_n[:], ps_stat[0:1, 0:CH], 1.0 / D)
            stats2 = cp_.tile([1, 2 * CH], F32)
            nc.vector.tensor_copy(stats2[:], ps_stat[:])

            # transpose stats to [128, 4, 2] for per-partition rstd
            statsT = cp_.tile([128, 4, 2], F32)
            for s in range(2):
                for tq in range(4):
                    ptr2 = ptrp.tile([128, 128], F32, tag="ptr")
                    nc.tensor.transpose(
                        ptr2[:, 0:1], stats2[0:1, CH * s + 128 * tq : CH * s + 128 * tq + 128],
                        identf[0:1, 0:1],
                    )
                    nc.vector.tensor_copy(statsT[:, tq, s : s + 1], ptr2[:, 0:1])
            muT = cp_.tile([128, 4], F32)
            nc.vector.tensor_scalar_mul(muT[:], statsT[:, :, 0], 1.0 / D)
            varT = cp_.tile([128, 4], F32)
            nc.vector.tensor_scalar_mul(varT[:], statsT[:, :, 1], 1.0 / D)
            mu2T = cp_.tile([128, 4], F32)
            nc.vector.tensor_mul(mu2T[:], muT[:], muT[:])
            nc.vector.tensor_sub(varT[:], varT[:], mu2T[:])
            sdT = cp_.tile([128, 4], F32)
            nc.scalar.activation(sdT[:], varT[:], AF.Sqrt, bias=epsb[:], scale=1.0)
            rstdT = cp_.tile([128, 4], F32)
            nc.vector.reciprocal(rstdT[:], sdT[:])

            # ---- (h) out = rstd*(center^T @ Wo') + res ----
            for tt in range(4):
                pho = pmm.tile([128, D], F32, tag="pmm")
                for ee in range(4):
                    nc.tensor.matmul(
                        pho[:], retrT[:, ee, ts(tt, 128)], wo_sb[:, ee, :],
                        start=(ee == 0), stop=False,
                    )
                nc.tensor.matmul(
                    pho[:], mu_n[0:1, ts(tt, 128)], ncs_sb[:],
                    start=False, stop=True,
                )
                out_t = rot4.tile([128, D], F32, tag="outt")
                nc.vector.scalar_tensor_tensor(
                    out=out_t[:], in0=pho[:], scalar=rstdT[:, tt : tt + 1],
                    in1=res_sb[:, tt, :], op0=ALU.mult, op1=ALU.add,
                )
                nc.sync.dma_start(y[ts(tt, 128), :], out_t[:])

    nc.compile()
    return nc


def _get_nc(zero_bv: bool):
    key = ("nc", zero_bv)
    if key not in _NC_CACHE:
        _NC_CACHE[key] = _build(zero_bv)
    return _NC_CACHE[key]


def _prep_inputs(inputs):
    x = np.asarray(inputs["x"], np.float32)
    W1 = np.asarray(inputs["W1"], np.float32)
    b1 = np.asarray(inputs["b1"], np.float32)
    W2 = np.asarray(inputs["W2"], np.float32)
    b2 = np.asarray(inputs["b2"], np.float32)
    pos_scale = float(np.asarray(inputs["pos_scale"]).reshape(-1)[0])
    content_scale = float(np.asarray(inputs["content_scale"]).reshape(-1)[0])
    Wv = np.asarray(inputs["Wv"], np.float32)
    bv = np.asarray(inputs["bv"], np.float32)
    ln_g = np.asarray(inputs["ln_g"], np.float32)
    ln_b = np.asarray(inputs["ln_b"], np.float32)
    Wo = np.asarray(inputs["Wo"], np.float32)
    bo = np.asarray(inputs["bo"], np.float32)

    Wop = ln_g[:, None] * Wo                       # fold ln gain
    ncs_v = -Wop.sum(axis=0, dtype=np.float64).astype(np.float32)[None, :]
    res_base = (ln_b @ Wo + bo).astype(np.float32)  # fold ln bias + out bias

    # [p, k, out]: row Din = 128k+p  (exact SBUF layout, contiguous DMA)
    w1_t = np.ascontiguousarray(W1.reshape(4, 128, D).transpose(1, 0, 2))
    w2_t = np.ascontiguousarray(W2.reshape(4, 128, K).transpose(1, 0, 2))
    wv_t = np.ascontiguousarray(Wv.reshape(4, 128, D).transpose(1, 0, 2))
    wo_t = np.ascontiguousarray(Wop.reshape(4, 128, D).transpose(1, 0, 2))
    b1p = np.ascontiguousarray(b1.reshape(4, 128).T)
    b2p = np.tile(b2, 4)[:, None].astype(np.float32)
    kc = np.full((128, 1), np.pi * content_scale, np.float32)
    bvr = bv[None, :].astype(np.float32)

    freqs = 1.0 / (10000.0 ** (np.arange(K, dtype=np.float64) / K))

    in_maps = []
    for core in range(NCORE):
        b, i = divmod(core, 4)
        pad = 1536 - 512 * i
        nreal = 512 * (i + 1)
        xpad = np.zeros((T, D), np.float32)
        xpad[pad:] = x[b, :nreal]
        # xt dram layout: [c, 128, 4, CH]: [p, k] = Din 128k+p, per-chunk contiguous
        xt = np.ascontiguousarray(
            xpad.T.reshape(4, 128, 4, CH).transpose(2, 1, 0, 3))

        lidx = np.arange(T, dtype=np.float64) - pad
        ang = pos_scale * lidx[:, None] * freqs[None, :]      # [T, K]
        # S path: sin(ang + ct) -> fold ang = ps + pi*n, ps in [-pi/2, pi/2]
        n_s = np.round(ang / np.pi)
        ps_f = (ang - np.pi * n_s).astype(np.float32)
        sg_s = np.where(n_s % 2 == 0, 1.0, -1.0).astype(np.float32)
        # C path: cos(ang + ct) = sin(pi/2 + ang + ct)
        n_c = np.round((ang + np.pi / 2) / np.pi)
        pc_f = (ang + np.pi / 2 - np.pi * n_c).astype(np.float32)
        sg_c = np.where(n_c % 2 == 0, 1.0, -1.0).astype(np.float32)
        # padded rows contribute nothing: zero the signs (C = S = 0)
        sg_s[lidx < 0] = 0.0
        sg_c[lidx < 0] = 0.0
        ps_f[lidx < 0] = 0.0
        pc_f[lidx < 0] = 0.0

        def pack(a):
            return np.ascontiguousarray(
                a.reshape(4, CH, K).transpose(0, 2, 1).reshape(128, CH))

        pcos = pack(pc_f)
        psin = pack(ps_f)
        sgcp = pack(sg_c)
        sgsp = pack(sg_s)

        resc = (x[b, 512 * i : 512 * i + CH] + res_base[None, :]).astype(np.float32)

        xnat = np.ascontiguousarray(
            xpad[0:1536].reshape(12, 128, D).transpose(1, 0, 2))
        xdt = ml_dtypes.bfloat16 if USE_BF16 else np.float32
        in_maps.append({
            "xt": xt.astype(xdt), "xn": xnat.astype(xdt),
            "w1f": np.ascontiguousarray(w1_t[:, :, 0:128].astype(xdt)),
            "w1": w1_t.astype(xdt), "w2": w2_t.astype(xdt),
            "wv": wv_t.astype(xdt), "wo": wo_t.astype(xdt),
            "ncs": ncs_v.astype(xdt), "bvr": bvr.astype(xdt),
            "b1p": b1p, "b2p": b2p,
            "pcos": pcos, "psin": psin, "sgc": sgcp, "sgs": sgsp,
            "res": np.ascontiguousarray(resc.reshape(4, 128, D).transpose(1, 0, 2)),
            "kc": kc,
        })
    return in_maps


def kernel(**inputs) -> np.ndarray:
    global LAST_RESULT
    zero_bv = bool(np.all(np.asarray(inputs["bv"]) == 0.0))
    nc = _get_nc(zero_bv)
    in_maps = _prep_inputs(inputs)
    result = run_bass_kernel_spmd(
        nc, in_maps, core_ids=list(range(NCORE)), **RUN_KWARGS
    )
    LAST_RESULT = result
    y = np.empty((B, L, D), np.float32)
    for core in range(NCORE):
        b, i = divmod(core, 4)
        y[b, 512 * i : 512 * i + CH] = result.results[core]["y"]
    return y



# revision 2
# speedup vs baseline: 1.1692x; 1.1692x over previous
"""Trainium2 Bass kernel for nn_DualAddressingPhasor.

Math: the phasor cumsum-bind/retrieve is causal linear attention:
  retrieved[l] = sum_{l'<=l} (sum_k cos(phi_l,k - phi_l',k)) * value[l']
Per 512-row chunk this is (1) a carried state [2K, D] = CS^T @ value over
the prefix plus (2) intra-chunk attention triu(Cc@Cc^T + Sc@Sc^T) @ value_c.

Sharding: 8 cores = 2 batches x 4 sequence chunks of 512. Uniform SPMD
program; per-core variation is entirely in the data (right-aligned
zero-padded prefix, host-precomputed positional sin/cos with zeros in the
padding so padded rows contribute nothing).
"""

import sys

for _p in ("/opt/trn_rl_repo",):
    if _p not in sys.path:
        sys.path.append(_p)

import numpy as np
import ml_dtypes

import concourse.bacc as bacc
import concourse.tile as tile
import concourse.mybir as mybir
from concourse.bass import ts
from concourse.bass_utils import run_bass_kernel_spmd
from concourse.masks import make_identity

F32 = mybir.dt.float32
F32R = mybir.dt.float32r
BF16 = mybir.dt.bfloat16
USE_BF16 = True
AF = mybir.ActivationFunctionType
ALU = mybir.AluOpType

D = 512
K = 32
B = 2
L = 2048
CH = 512          # chunk rows per core
T = 2048          # padded rows processed per core
NCORE = 8
HALF_PI = float(np.pi / 2)

_NC_CACHE = {}
LAST_RESULT = None
RUN_KWARGS = {}


def _build(zero_bv: bool):
    XDT = BF16 if USE_BF16 else F32R
    nc = bacc.Bacc("TRN2", num_devices=NCORE)

    xt = nc.dram_tensor("xt", [4, 128, 4, CH], XDT, kind="ExternalInput")
    w1f = nc.dram_tensor("w1f", [128, 4, 128], XDT, kind="ExternalInput")
    w1 = nc.dram_tensor("w1", [128, 4, D], XDT, kind="ExternalInput")
    w2 = nc.dram_tensor("w2", [128, 4, K], XDT, kind="ExternalInput")
    wv = nc.dram_tensor("wv", [128, 4, D], XDT, kind="ExternalInput")
    wo = nc.dram_tensor("wo", [128, 4, D], XDT, kind="ExternalInput")
    ncs = nc.dram_tensor("ncs", [1, D], XDT, kind="ExternalInput")
    bvr = nc.dram_tensor("bvr", [1, D], XDT, kind="ExternalInput")
    b1p = nc.dram_tensor("b1p", [128, 4], F32, kind="ExternalInput")
    b2p = nc.dram_tensor("b2p", [128, 1], F32, kind="ExternalInput")
    pcos = nc.dram_tensor("pcos", [128, CH], F32, kind="ExternalInput")
    psin = nc.dram_tensor("psin", [128, CH], F32, kind="ExternalInput")
    sgc = nc.dram_tensor("sgc", [128, CH], F32, kind="ExternalInput")
    sgs = nc.dram_tensor("sgs", [128, CH], F32, kind="ExternalInput")
    res = nc.dram_tensor("res", [128, 4, D], F32, kind="ExternalInput")
    xn = nc.dram_tensor("xn", [128, 12, D], XDT, kind="ExternalInput")
    y = nc.dram_tensor("y", [CH, D], F32, kind="ExternalOutput")

    kc = nc.dram_tensor("kc", [128, 1], F32, kind="ExternalInput")  # pi*content_scale

    with tile.TileContext(nc) as tc:
        with (
            tc.tile_pool(name="const", bufs=1) as cp_,
            tc.tile_pool(name="big", bufs=1) as bigp,
            tc.tile_pool(name="rot", bufs=2) as rot,
            tc.tile_pool(name="rot4", bufs=4) as rot4,
            tc.tile_pool(name="pmm", bufs=4, space="PSUM") as pmm,
            tc.tile_pool(name="pone", bufs=1, space="PSUM") as pone,
            tc.tile_pool(name="ptr", bufs=2, space="PSUM") as ptrp,
        ):
            # ---- input loads, ordered so the first (a) matmuls start ASAP ----
            xt_sb = bigp.tile([128, 4, T], XDT)
            w1_sb = cp_.tile([128, 4, D], XDT)
            w2_sb = cp_.tile([128, 4, K], XDT)
            wv_sb = cp_.tile([128, 4, D], XDT)
            wo_sb = cp_.tile([128, 4, D], XDT)
            res_sb = cp_.tile([128, 4, D], F32)
            bvr_sb = cp_.tile([1, D], XDT)
            b1p_sb = cp_.tile([128, 4], F32)
            b2p_sb = cp_.tile([128, 1], F32)
            kc_sb = cp_.tile([128, 1], F32)
            pcos_sb = cp_.tile([128, CH], F32)
            psin_sb = cp_.tile([128, CH], F32)
            ncs_sb = cp_.tile([1, D], XDT)

            # early-need: xt on sync, w1/w2/wv on gpsimd, small aux on scalar.
            # late-need (xn, wo, res) go behind xt on the sync queue so they
            # don't steal HBM bandwidth from the (a) matmuls.
            # first-needed pair: dout0 slice of w1 + chunk0 of xt (2 descriptors)
            nc.sync.dma_start(w1_sb[:, :, 0:128], w1f[:])
            nc.gpsimd.dma_start(xt_sb[:, :, ts(0, CH)], xt[0])
            nc.sync.dma_start(w1_sb[:, :, 128:512], w1[:, :, 128:512])
            nc.gpsimd.dma_start(xt_sb[:, :, ts(1, CH)], xt[1])
            nc.sync.dma_start(xt_sb[:, :, ts(2, CH)], xt[2])
            nc.sync.dma_start(xt_sb[:, :, ts(3, CH)], xt[3])
            nc.gpsimd.dma_start(w2_sb[:], w2[:])
            nc.gpsimd.dma_start(wv_sb[:], wv[:])
            nc.scalar.dma_start(b1p_sb[:], b1p[:])
            nc.scalar.dma_start(b2p_sb[:], b2p[:])
            nc.scalar.dma_start(kc_sb[:], kc[:])
            nc.scalar.dma_start(pcos_sb[:], pcos[:])
            nc.scalar.dma_start(psin_sb[:], psin[:])
            sgc_sb = cp_.tile([128, CH], F32)
            nc.scalar.dma_start(sgc_sb[:], sgc[:])
            sgs_sb = cp_.tile([128, CH], F32)
            nc.scalar.dma_start(sgs_sb[:], sgs[:])
            nc.scalar.dma_start(bvr_sb[:], bvr[:])
            xn_sb = bigp.tile([128, 12, D], XDT)
            nc.sync.dma_start(xn_sb[:], xn[:])
            nc.gpsimd.dma_start(wo_sb[:], wo[:])
            nc.sync.dma_start(res_sb[:], res[:])
            nc.scalar.dma_start(ncs_sb[:], ncs[:])

            onesf = cp_.tile([128, 128], F32)
            nc.vector.memset(onesf[:], 1.0)
            onesr = cp_.tile([1, 128], XDT)
            nc.vector.tensor_copy(onesr[:], onesf[0:1, :])
            onesc = cp_.tile([128, 1], XDT)
            nc.vector.tensor_copy(onesc[:], onesf[:, 0:1])
            onescol_x = onesc
            halfpi = cp_.tile([128, 1], F32)
            nc.vector.memset(halfpi[:], HALF_PI)
            epsb = cp_.tile([128, 1], F32)
            nc.vector.memset(epsb[:], 1e-5)

            identf = cp_.tile([128, 128], F32)
            make_identity(nc, identf[:])
            identr = cp_.tile([128, 128], F32R)
            nc.vector.tensor_copy(identr[:], identf[:])

            # triangular masks for intra-chunk causal attention (lhsT form:
            # tri[p, tr, y] = 1 iff y >= p + 128*tr)
            tri = cp_.tile([128, 4, CH], F32)
            for tr in range(4):
                nc.gpsimd.memset(tri[:, tr, :], 0.0)
                nc.gpsimd.affine_select(
                    out=tri[:, tr, :], in_=tri[:, tr, :],
                    compare_op=ALU.is_gt, fill=1.0, base=128 * tr,
                    pattern=[[-1, CH]], channel_multiplier=1,
                )

            # ---- (a) h^T = tanh(W1^T x^T + b1), chunked; (b) packed content phase ----
            tt_sb = cp_.tile([128, CH], F32)
            for c in range(4):
                h_ck = rot.tile([128, 4, CH], XDT, tag="hck")
                for dout in range(4):
                    ph = pmm.tile([128, CH], F32, tag="pmm")
                    for k in range(4):
                        nc.tensor.matmul(
                            ph[:], w1_sb[:, k, ts(dout, 128)],
                            xt_sb[:, k, ts(c, CH)],
                            start=(k == 0), stop=(k == 3),
                        )
                    nc.scalar.activation(
                        h_ck[:, dout, :], ph[:], AF.Tanh,
                        bias=b1p_sb[:, dout : dout + 1], scale=1.0,
                    )
                pc = pmm.tile([32, CH], F32, tag="pmm")
                for k in range(4):
                    nc.tensor.matmul(
                        pc[:], w2_sb[:, k, :], h_ck[:, k, :],
                        start=(k == 0), stop=(k == 3),
                    )
                nc.scalar.activation(
                    tt_sb[32 * c : 32 * c + 32, :], pc[:], AF.Tanh,
                    bias=b2p_sb[0:32, :], scale=1.0,
                )

            # ---- phases: S = sgn_s*sin(ps + kc*t), C = sgn_c*sin(pc2 + kc*t)
            # (host folds pos phase to [-pi/2, pi/2] plus a sign; |kc*t|<=pi/2
            #  keeps the Sin LUT inside its accurate [-pi, pi] window)
            phS = cp_.tile([128, CH], F32)
            nc.vector.scalar_tensor_tensor(
                out=phS[:], in0=tt_sb[:], scalar=kc_sb[:], in1=psin_sb[:],
                op0=ALU.mult, op1=ALU.add,
            )
            s0 = cp_.tile([128, CH], F32)
            nc.scalar.activation(s0[:], phS[:], AF.Sin)
            spk = cp_.tile([128, CH], F32R)
            nc.vector.tensor_mul(spk[:], s0[:], sgs_sb[:])
            phC = cp_.tile([128, CH], F32)
            nc.vector.scalar_tensor_tensor(
                out=phC[:], in0=tt_sb[:], scalar=kc_sb[:], in1=pcos_sb[:],
                op0=ALU.mult, op1=ALU.add,
            )
            c0 = cp_.tile([128, CH], F32)
            nc.scalar.activation(c0[:], phC[:], AF.Sin)
            cpk = cp_.tile([128, CH], F32R)
            nc.vector.tensor_mul(cpk[:], c0[:], sgc_sb[:])

            # combined chunk CS tile [64, CH] at base 0 (C rows 0:32, S 32:64)
            csc = cp_.tile([64, CH], F32R)
            nc.vector.tensor_copy(csc[0:32, :], cpk[96:128, :])
            nc.vector.tensor_copy(csc[32:64, :], spk[96:128, :])

            # ---- (c) value = x @ Wv + bv for the own chunk only ----
            value_sb = bigp.tile([128, 4, D], F32R)
            for tt in range(4):
                pv = pmm.tile([128, D], F32, tag="pmm")
                for k in range(4):
                    nc.tensor.matmul(
                        pv[:], xt_sb[:, k, ts(12 + tt, 128)], wv_sb[:, k, :],
                        start=(k == 0), stop=(zero_bv and k == 3),
                    )
                if not zero_bv:
                    nc.tensor.matmul(pv[:], onesr[:], bvr_sb[:], start=False, stop=True)
                if tt % 2 == 0:
                    nc.vector.tensor_copy(value_sb[:, tt, :], pv[:])
                else:
                    nc.scalar.copy(value_sb[:, tt, :], pv[:])

            # ---- transpose C/S to natural layout for the state matmul ----
            # csm[p, c, b, j]: natural row l = 512c + 128b + p, freq j (0:32 C, 32:64 S)
            csm = cp_.tile([128, 4, 4, 64], XDT)
            for src, j0 in ((cpk, 0), (spk, 32)):
                for bb in range(4):
                    ptr = ptrp.tile([128, 128], F32R, tag="ptr")
                    nc.tensor.transpose(ptr[:], src[:, ts(bb, 128)], identr[:])
                    nc.vector.tensor_copy(
                        csm[:, :, bb, j0 : j0 + 32],
                        ptr[:].rearrange("p (c j) -> p c j", j=32),
                    )

            # ---- (d) prefix state = (CSm^T X) @ Wv  (+ msum*bv) ----
            pg = pone.tile([64, D], F32, tag="pst")
            for kt in range(12):
                c, bb = kt // 4, kt % 4
                nc.tensor.matmul(
                    pg[:], csm[:, c, bb, :], xn_sb[:, kt, :],
                    start=(kt == 0), stop=(kt == 11),
                )
            g_sb = cp_.tile([64, D], F32R)
            nc.vector.tensor_copy(g_sb[:], pg[:])
            gt_sb = cp_.tile([128, 4, 64], XDT)
            for kk in range(4):
                ptg = ptrp.tile([128, 128], F32R, tag="ptr")
                nc.tensor.transpose(
                    ptg[0:128, 0:64], g_sb[:, ts(kk, 128)], identr[0:64, 0:64]
                )
                nc.vector.tensor_copy(gt_sb[:, kk, :], ptg[:, 0:64])
            pst = pone.tile([64, D], F32, tag="pst")
            for kk in range(4):
                nc.tensor.matmul(
                    pst[:], gt_sb[:, kk, :], wv_sb[:, kk, :],
                    start=(kk == 0), stop=(zero_bv and kk == 3),
                )
            if not zero_bv:
                # msum[j] = sum_l CSm[l, j]; state += msum (x) bv
                pms = ptrp.tile([64, 1], F32, tag="ptr")
                for kt in range(12):
                    c, bb = kt // 4, kt % 4
                    nc.tensor.matmul(
                        pms[:], csm[:, c, bb, :], onescol_x[:],
                        start=(kt == 0), stop=(kt == 11),
                    )
                ms_sb = cp_.tile([64, 1], XDT)
                nc.vector.tensor_copy(ms_sb[:], pms[:])
                msT = cp_.tile([1, 64], XDT)
                ptm = ptrp.tile([128, 128], F32R, tag="ptr")
                nc.tensor.transpose(ptm[0:1, 0:64], ms_sb[:], identr[0:64, 0:64])
                nc.vector.tensor_copy(msT[:], ptm[0:1, 0:64])
                nc.tensor.matmul(pst[:], msT[:], bvr_sb[:], start=False, stop=True)
            state_sb = cp_.tile([64, D], F32R)
            nc.vector.tensor_copy(state_sb[:], pst[:])

            # ---- (e) intra-chunk scores, triu-masked ----
            p_sb = cp_.tile([128, 4, CH], F32R)
            for tr in range(4):
                psc = pmm.tile([128, CH], F32, tag="pmm")
                nc.tensor.matmul(
                    psc[:], csc[:, ts(tr, 128)], csc[:],
                    start=True, stop=True,
                )
                nc.vector.tensor_mul(p_sb[:, tr, :], psc[:], tri[:, tr, :])

            # ---- (f)+(g) retrieved^T [D, CH] ----
            retrT = cp_.tile([128, 4, CH], XDT)
            sq_sb = cp_.tile([128, 4, CH], XDT)
            for dd in range(4):
                pr = pmm.tile([128, CH], F32, tag="pmm")
                for tr in range(4):
                    nc.tensor.matmul(
                        pr[:], value_sb[:, tr, ts(dd, 128)], p_sb[:, tr, :],
                        start=(tr == 0), stop=False,
                    )
                nc.tensor.matmul(
                    pr[:], state_sb[:, ts(dd, 128)], csc[:],
                    start=False, stop=True,
                )
                nc.vector.tensor_copy(retrT[:, dd, :], pr[:])
                nc.scalar.square(sq_sb[:, dd, :], pr[:])

            # ---- LayerNorm stats (feature dim = partitions here) ----
            ps_stat = pone.tile([1, 2 * CH], F32, tag="pst")
            for dd in range(4):
                nc.tensor.matmul(
                    ps_stat[0:1, 0:CH], onesc[:], retrT[:, dd, :],
                    start=(dd == 0), stop=(dd == 3),
                )
            for dd in range(4):
                nc.tensor.matmul(
                    ps_stat[0:1, CH : 2 * CH], onesc[:], sq_sb[:, dd, :],
                    start=(dd == 0), stop=(dd == 3),
                )
            mu_n = cp_.tile([1, CH], XDT)
            nc.vector.tensor_scalar_mul(mu_n[:], ps_stat[0:1, 0:CH], 1.0 / D)
            stats2 = cp_.tile([1, 2 * CH], F32)
            nc.vector.tensor_copy(stats2[:], ps_stat[:])

            # transpose stats to [128, 4, 2] for per-partition rstd
            statsT = cp_.tile([128, 4, 2], F32)
            for s in range(2):
                for tq in range(4):
                    ptr2 = ptrp.tile([128, 128], F32, tag="ptr")
                    nc.tensor.transpose(
                        ptr2[:, 0:1], stats2[0:1, CH * s + 128 * tq : CH * s + 128 * tq + 128],
                        identf[0:1, 0:1],
                    )
                    nc.vector.tensor_copy(statsT[:, tq, s : s + 1], ptr2[:, 0:1])
            muT = cp_.tile([128, 4], F32)
            nc.vector.tensor_scalar_mul(muT[:], statsT[:, :, 0], 1.0 / D)
            varT = cp_.tile([128, 4], F32)
            nc.vector.tensor_scalar_mul(varT[:], statsT[:, :, 1], 1.0 / D)
            mu2T = cp_.tile([128, 4], F32)
            nc.vector.tensor_mul(mu2T[:], muT[:], muT[:])
            nc.vector.tensor_sub(varT[:], varT[:], mu2T[:])
            sdT = cp_.tile([128, 4], F32)
            nc.scalar.activation(sdT[:], varT[:], AF.Sqrt, bias=epsb[:], scale=1.0)
            rstdT = cp_.tile([128, 4], F32)
            nc.vector.reciprocal(rstdT[:], sdT[:])

            # ---- (h) out = rstd*(center^T @ Wo') + res ----
            for tt in range(4):
                pho = pmm.tile([128, D], F32, tag="pmm")
                for ee in range(4):
                    nc.tensor.matmul(
                        pho[:], retrT[:, ee, ts(tt, 128)], wo_sb[:, ee, :],
                        start=(ee == 0), stop=False,
                    )
                nc.tensor.matmul(
                    pho[:], mu_n[0:1, ts(tt, 128)], ncs_sb[:],
                    start=False, stop=True,
                )
                out_t = rot4.tile([128, D], F32, tag="outt")
                nc.vector.scalar_tensor_tensor(
                    out=out_t[:], in0=pho[:], scalar=rstdT[:, tt : tt + 1],
                    in1=res_sb[:, tt, :], op0=ALU.mult, op1=ALU.add,
                )
                nc.sync.dma_start(y[ts(tt, 128), :], out_t[:])

    nc.compile()
    return nc


def _get_nc(zero_bv: bool):
    key = ("nc", zero_bv)
    if key not in _NC_CACHE:
        _NC_CACHE[key] = _build(zero_bv)
    return _NC_CACHE[key]


def _prep_inputs(inputs):
    x = np.asarray(inputs["x"], np.float32)
    W1 = np.asarray(inputs["W1"], np.float32)
    b1 = np.asarray(inputs["b1"], np.float32)
    W2 = np.asarray(inputs["W2"], np.float32)
    b2 = np.asarray(inputs["b2"], np.float32)
    pos_scale = float(np.asarray(inputs["pos_scale"]).reshape(-1)[0])
    content_scale = float(np.asarray(inputs["content_scale"]).reshape(-1)[0])
    Wv = np.asarray(inputs["Wv"], np.float32)
    bv = np.asarray(inputs["bv"], np.float32)
    ln_g = np.asarray(inputs["ln_g"], np.float32)
    ln_b = np.asarray(inputs["ln_b"], np.float32)
    Wo = np.asarray(inputs["Wo"], np.float32)
    bo = np.asarray(inputs["bo"], np.float32)

    Wop = ln_g[:, None] * Wo                       # fold ln gain
    ncs_v = -Wop.sum(axis=0, dtype=np.float64).astype(np.float32)[None, :]
    res_base = (ln_b @ Wo + bo).astype(np.float32)  # fold ln bias + out bias

    # [p, k, out]: row Din = 128k+p  (exact SBUF layout, contiguous DMA)
    w1_t = np.ascontiguousarray(W1.reshape(4, 128, D).transpose(1, 0, 2))
    w2_t = np.ascontiguousarray(W2.reshape(4, 128, K).transpose(1, 0, 2))
    wv_t = np.ascontiguousarray(Wv.reshape(4, 128, D).transpose(1, 0, 2))
    wo_t = np.ascontiguousarray(Wop.reshape(4, 128, D).transpose(1, 0, 2))
    b1p = np.ascontiguousarray(b1.reshape(4, 128).T)
    b2p = np.tile(b2, 4)[:, None].astype(np.float32)
    kc = np.full((128, 1), np.pi * content_scale, np.float32)
    bvr = bv[None, :].astype(np.float32)

    freqs = 1.0 / (10000.0 ** (np.arange(K, dtype=np.float64) / K))

    in_maps = []
    for core in range(NCORE):
        b, i = divmod(core, 4)
        pad = 1536 - 512 * i
        nreal = 512 * (i + 1)
        xpad = np.zeros((T, D), np.float32)
        xpad[pad:] = x[b, :nreal]
        # xt dram layout: [c, 128, 4, CH]: [p, k] = Din 128k+p, per-chunk contiguous
        xt = np.ascontiguousarray(
            xpad.T.reshape(4, 128, 4, CH).transpose(2, 1, 0, 3))

        lidx = np.arange(T, dtype=np.float64) - pad
        ang = pos_scale * lidx[:, None] * freqs[None, :]      # [T, K]
        # S path: sin(ang + ct) -> fold ang = ps + pi*n, ps in [-pi/2, pi/2]
        n_s = np.round(ang / np.pi)
        ps_f = (ang - np.pi * n_s).astype(np.float32)
        sg_s = np.where(n_s % 2 == 0, 1.0, -1.0).astype(np.float32)
        # C path: cos(ang + ct) = sin(pi/2 + ang + ct)
        n_c = np.round((ang + np.pi / 2) / np.pi)
        pc_f = (ang + np.pi / 2 - np.pi * n_c).astype(np.float32)
        sg_c = np.where(n_c % 2 == 0, 1.0, -1.0).astype(np.float32)
        # padded rows contribute nothing: zero the signs (C = S = 0)
        sg_s[lidx < 0] = 0.0
        sg_c[lidx < 0] = 0.0
        ps_f[lidx < 0] = 0.0
        pc_f[lidx < 0] = 0.0

        def pack(a):
            return np.ascontiguousarray(
                a.reshape(4, CH, K).transpose(0, 2, 1).reshape(128, CH))

        pcos = pack(pc_f)
        psin = pack(ps_f)
        sgcp = pack(sg_c)
        sgsp = pack(sg_s)

        resc = (x[b, 512 * i : 512 * i + CH] + res_base[None, :]).astype(np.float32)

        xnat = np.ascontiguousarray(
            xpad[0:1536].reshape(12, 128, D).transpose(1, 0, 2))
        xdt = ml_dtypes.bfloat16 if USE_BF16 else np.float32
        in_maps.append({
            "xt": xt.astype(xdt), "xn": xnat.astype(xdt),
            "w1f": np.ascontiguousarray(w1_t[:, :, 0:128].astype(xdt)),
            "w1": w1_t.astype(xdt), "w2": w2_t.astype(xdt),
            "wv": wv_t.astype(xdt), "wo": wo_t.astype(xdt),
            "ncs": ncs_v.astype(xdt), "bvr": bvr.astype(xdt),
            "b1p": b1p, "b2p": b2p,
            "pcos": pcos, "psin": psin, "sgc": sgcp, "sgs": sgsp,
            "res": np.ascontiguousarray(resc.reshape(4, 128, D).transpose(1, 0, 2)),
            "kc": kc,
        })
    return in_maps


def kernel(**inputs) -> np.ndarray:
    global LAST_RESULT
    zero_bv = bool(np.all(np.asarray(inputs["bv"]) == 0.0))
    nc = _get_nc(zero_bv)
    in_maps = _prep_inputs(inputs)
    result = run_bass_kernel_spmd(
        nc, in_maps, core_ids=list(range(NCORE)), **RUN_KWARGS
    )
    LAST_RESULT = result
    y = np.empty((B, L, D), np.float32)
    for core in range(NCORE):
        b, i = divmod(core, 4)
        y[b, 512 * i : 512 * i + CH] = result.results[core]["y"]
    return y



# revision 6
# speedup vs baseline: 1.2063x; 1.0318x over previous
"""Trainium2 Bass kernel for nn_DualAddressingPhasor.

Math: the phasor cumsum-bind/retrieve is causal linear attention:
  retrieved[l] = sum_{l'<=l} (sum_k cos(phi_l,k - phi_l',k)) * value[l']
Per 512-row chunk this is (1) a carried state [2K, D] = CS^T @ value over
the prefix plus (2) intra-chunk attention triu(Cc@Cc^T + Sc@Sc^T) @ value_c.

Sharding: 8 cores = 2 batches x 4 sequence chunks of 512. Uniform SPMD
program; per-core variation is entirely in the data (right-aligned
zero-padded prefix, host-precomputed positional phase tables with zero
signs in the padding so padded rows contribute nothing).

v2: all-bf16 matmul operands (fp32r matmuls pay a serial ~70ns weight
load per matmul; bf16 hits the 216ns/512-row peak), natural-layout
phases derived by transposing the content tile tt (4 transposes instead
of 8), LayerNorm rstd computed in row space with a DMA round-trip
through DRAM to transpose [1,512]->[128,4] (replaces 8 PE transposes),
stats matmuls interleaved with the retrieve matmuls, and a DMA plan
that spreads x across queues so the PE starts early and never stalls.
"""

import sys

for _p in ("/opt/trn_rl_repo",):
    if _p not in sys.path:
        sys.path.append(_p)

import numpy as np
import ml_dtypes

import concourse.bacc as bacc
import concourse.tile as tile
import concourse.mybir as mybir
from concourse.bass import ts
from concourse.bass_utils import run_bass_kernel_spmd
from concourse.masks import make_identity

F32 = mybir.dt.float32
BF16 = mybir.dt.bfloat16
AF = mybir.ActivationFunctionType
ALU = mybir.AluOpType

D = 512
K = 32
B = 2
L = 2048
CH = 512          # chunk rows per core
T = 2048          # padded rows processed per core
NCORE = 8

_NC_CACHE = {}
LAST_RESULT = None
RUN_KWARGS = {}


def _build(zero_bv: bool):
    nc = bacc.Bacc("TRN2", num_devices=NCORE)

    xt = nc.dram_tensor("xt", [4, 128, 4, CH], BF16, kind="ExternalInput")
    w1f = nc.dram_tensor("w1f", [128, 4, 128], BF16, kind="ExternalInput")
    w1 = nc.dram_tensor("w1", [128, 4, D], BF16, kind="ExternalInput")
    w2 = nc.dram_tensor("w2", [128, 4, K], BF16, kind="ExternalInput")
    wv = nc.dram_tensor("wv", [128, 4, D], BF16, kind="ExternalInput")
    wo = nc.dram_tensor("wo", [128, 4, D], BF16, kind="ExternalInput")
    ncs = nc.dram_tensor("ncs", [1, D], BF16, kind="ExternalInput")
    bvr = nc.dram_tensor("bvr", [1, D], BF16, kind="ExternalInput")
    b1p = nc.dram_tensor("b1p", [128, 4], F32, kind="ExternalInput")
    b2p = nc.dram_tensor("b2p", [128, 1], F32, kind="ExternalInput")
    kc = nc.dram_tensor("kc", [128, 1], F32, kind="ExternalInput")
    tblN = nc.dram_tensor("tblN", [128, 2, 4, 128], F32, kind="ExternalInput")
    sgnN = nc.dram_tensor("sgnN", [128, 2, 4, 128], BF16, kind="ExternalInput")
    tblF = nc.dram_tensor("tblF", [64, CH], F32, kind="ExternalInput")
    sgnF = nc.dram_tensor("sgnF", [64, CH], BF16, kind="ExternalInput")
    epsn = nc.dram_tensor("epsn", [1, CH], F32, kind="ExternalInput")
    res = nc.dram_tensor("res", [128, 4, D], F32, kind="ExternalInput")
    xn = nc.dram_tensor("xn", [128, 12, D], BF16, kind="ExternalInput")
    y = nc.dram_tensor("y", [CH, D], F32, kind="ExternalOutput")

    with tile.TileContext(nc) as tc:
        with (
            tc.tile_pool(name="const", bufs=1) as cp_,
            tc.tile_pool(name="big", bufs=1) as bigp,
            tc.tile_pool(name="rot", bufs=3) as rot,
            tc.tile_pool(name="rot4", bufs=4) as rot4,
            tc.tile_pool(name="pmm", bufs=4, space="PSUM") as pmm,
            tc.tile_pool(name="pone", bufs=1, space="PSUM") as pone,
            tc.tile_pool(name="ptr", bufs=2, space="PSUM") as ptrp,
            tc.tile_pool(name="dram", bufs=1, space="DRAM") as dram,
        ):
            # ---- input loads, spread over queues so (a) starts ASAP ----
            xt_sb = bigp.tile([128, 4, T], BF16)
            w1_sb = cp_.tile([128, 4, D], BF16)
            w2_sb = cp_.tile([128, 4, K], BF16)
            wv_sb = cp_.tile([128, 4, D], BF16)
            wo_sb = cp_.tile([128, 4, D], BF16)
            res_sb = cp_.tile([128, 4, D], F32)
            bvr_sb = cp_.tile([1, D], BF16)
            ncs_sb = cp_.tile([1, D], BF16)
            b1p_sb = cp_.tile([128, 4], F32)
            b2p_sb = cp_.tile([128, 1], F32)
            kc_sb = cp_.tile([128, 1], F32)
            tblN_sb = cp_.tile([128, 2, 4, 128], F32)
            sgnN_sb = cp_.tile([128, 2, 4, 128], BF16)
            tblF_sb = cp_.tile([64, CH], F32)
            sgnF_sb = cp_.tile([64, CH], BF16)
            epsn_sb = cp_.tile([1, CH], F32)
            xn_sb = bigp.tile([128, 12, D], BF16)

            # sync: first-need pair then mid-kernel needs
            nc.sync.dma_start(w1_sb[:, :, 0:128], w1f[:])
            nc.sync.dma_start(xt_sb[:, 0, ts(0, CH)], xt[0][:, 0, :])
            nc.sync.dma_start(xt_sb[:, 1:4, ts(0, CH)], xt[0][:, 1:4, :])
            nc.sync.dma_start(xt_sb[:, :, ts(2, CH)], xt[2])
            nc.sync.dma_start(xn_sb[:], xn[:])
            nc.sync.dma_start(tblN_sb[:], tblN[:])
            nc.sync.dma_start(sgnN_sb[:], sgnN[:])
            nc.sync.dma_start(ncs_sb[:], ncs[:])
            nc.sync.dma_start(bvr_sb[:], bvr[:])
            nc.sync.dma_start(epsn_sb[:], epsn[:])
            # gpsimd: w1 rest, then chunks 1/3, then late-need
            nc.gpsimd.dma_start(w1_sb[:, :, 128:512], w1[:, :, 128:512])
            nc.gpsimd.dma_start(xt_sb[:, :, ts(1, CH)], xt[1])
            nc.gpsimd.dma_start(xt_sb[:, :, ts(3, CH)], xt[3])
            nc.gpsimd.dma_start(w2_sb[:], w2[:])
            nc.gpsimd.dma_start(wv_sb[:], wv[:])
            nc.gpsimd.dma_start(tblF_sb[:], tblF[:])
            nc.gpsimd.dma_start(sgnF_sb[:], sgnF[:])
            nc.gpsimd.dma_start(res_sb[:], res[:])
            nc.gpsimd.dma_start(wo_sb[:], wo[:])
            # scalar: only the small early tables (keeps the act-table
            # loads near the queue head so the first tanh isn't delayed)
            nc.scalar.dma_start(b1p_sb[:], b1p[:])
            nc.scalar.dma_start(kc_sb[:], kc[:])
            nc.scalar.dma_start(b2p_sb[:], b2p[:])

            onesf = cp_.tile([128, 128], F32)
            nc.vector.memset(onesf[:], 1.0)
            onesr = cp_.tile([1, 128], BF16)
            nc.vector.tensor_copy(onesr[:], onesf[0:1, :])
            onesc = cp_.tile([128, 1], BF16)
            nc.vector.tensor_copy(onesc[:], onesf[:, 0:1])

            identb = cp_.tile([128, 128], BF16)
            make_identity(nc, identb[:])

            # triangular masks for intra-chunk causal attention (lhsT form:
            # tri[p, tr, y] = 1 iff y >= p + 128*tr)
            tri = cp_.tile([128, 4, CH], BF16)
            for tr in range(4):
                nc.gpsimd.memset(tri[:, tr, :], 0.0)
                nc.gpsimd.affine_select(
                    out=tri[:, tr, :], in_=tri[:, tr, :],
                    compare_op=ALU.is_gt, fill=1.0, base=128 * tr,
                    pattern=[[-1, CH]], channel_multiplier=1,
                )

            # ---- (a) h^T = tanh(W1^T x^T + b1) per chunk; (b) content tt,
            # deferred one chunk so the tanh latency hides under (a) ----
            tt_sb = cp_.tile([128, CH], BF16)
            h_cks = [None] * 4

            def emit_b(c):
                pc = pmm.tile([32, CH], F32, tag="pmm")
                for k in range(4):
                    nc.tensor.matmul(
                        pc[:], w2_sb[:, k, :], h_cks[c][:, k, :],
                        start=(k == 0), stop=(k == 3),
                    )
                nc.scalar.activation(
                    tt_sb[32 * c : 32 * c + 32, :], pc[:], AF.Tanh,
                    bias=b2p_sb[0:32, :], scale=1.0,
                )

            for c in range(4):
                h_ck = rot.tile([128, 4, CH], BF16, tag="hck")
                h_cks[c] = h_ck
                for dout in range(4):
                    ph = pmm.tile([128, CH], F32, tag="pmm")
                    for k in range(4):
                        nc.tensor.matmul(
                            ph[:], w1_sb[:, k, ts(dout, 128)],
                            xt_sb[:, k, ts(c, CH)],
                            start=(k == 0), stop=(k == 3),
                        )
                    nc.scalar.activation(
                        h_ck[:, dout, :], ph[:], AF.Tanh,
                        bias=b1p_sb[:, dout : dout + 1], scale=1.0,
                    )
                if c >= 1:
                    emit_b(c - 1)
            emit_b(3)

            # ---- freq-major phases for the own chunk (csc [64, CH]) ----
            # S/C = sgn * sin(tbl + kc*tt); host folds the positional part to
            # [-pi/2, pi/2] plus a sign so the Sin LUT stays accurate.
            ttF = cp_.tile([64, CH], BF16)
            nc.vector.tensor_copy(ttF[0:32, :], tt_sb[96:128, :])
            nc.vector.tensor_copy(ttF[32:64, :], tt_sb[96:128, :])
            argF = cp_.tile([64, CH], F32)
            nc.vector.scalar_tensor_tensor(
                out=argF[:], in0=ttF[:], scalar=kc_sb[0:64, :], in1=tblF_sb[:],
                op0=ALU.mult, op1=ALU.add,
            )
            sinF = cp_.tile([64, CH], F32)
            nc.scalar.activation(sinF[:], argF[:], AF.Sin)
            csc = cp_.tile([64, CH], BF16)
            nc.vector.tensor_mul(csc[:], sinF[:], sgnF_sb[:])

            # ---- (c) value = x @ Wv (+bv) for the own chunk ----
            value_sb = bigp.tile([128, 4, D], BF16)

            def emit_value(tt):
                pv = pmm.tile([128, D], F32, tag="pmm")
                for k in range(4):
                    nc.tensor.matmul(
                        pv[:], xt_sb[:, k, ts(12 + tt, 128)], wv_sb[:, k, :],
                        start=(k == 0), stop=(zero_bv and k == 3),
                    )
                if not zero_bv:
                    nc.tensor.matmul(pv[:], onesr[:], bvr_sb[:], start=False, stop=True)
                if tt % 2 == 0:
                    nc.vector.tensor_copy(value_sb[:, tt, :], pv[:])
                else:
                    nc.scalar.copy(value_sb[:, tt, :], pv[:])

            emit_value(0)
            emit_value(1)

            # ---- natural-layout phases: transpose tt, then sin per b-block ----
            # ttN[p, b, 32c+f] = tt[32c+f, 128b+p]; natural row l = 512c+128b+p
            ttN = cp_.tile([128, 4, 128], BF16)
            for b in range(4):
                ptr_ = ptrp.tile([128, 128], BF16, tag="ptr")
                nc.tensor.transpose(ptr_[:], tt_sb[:, ts(b, 128)], identb[:])
                nc.vector.tensor_copy(ttN[:, b, :], ptr_[:])

            emit_value(2)
            emit_value(3)

            argN = cp_.tile([128, 2, 4, 128], F32)
            for path in range(2):
                nc.vector.scalar_tensor_tensor(
                    out=argN[:, path], in0=ttN[:], scalar=kc_sb[:],
                    in1=tblN_sb[:, path], op0=ALU.mult, op1=ALU.add,
                )
            # csm2[p, b, c, path, f]: (path, f) contiguous so the pg lhsT
            # slice coalesces to a 2D [128, 64] access pattern
            csm2 = cp_.tile([128, 4, 4, 2, 32], BF16)
            sinN = cp_.tile([128, 2, 4, 128], F32)
            for b in range(4):
                nc.scalar.activation(sinN[:, :, b, :], argN[:, :, b, :], AF.Sin)
                for path in range(2):
                    nc.vector.tensor_mul(
                        csm2[:, b, :, path, :],
                        sinN[:, path, b, :].rearrange("p (c f) -> p c f", f=32),
                        sgnN_sb[:, path, b, :].rearrange("p (c f) -> p c f", f=32),
                    )

            # ---- (e) intra-chunk scores, triu-masked ----
            p_sb = cp_.tile([128, 4, CH], BF16)
            for tr in range(4):
                psc = pmm.tile([128, CH], F32, tag="pmm")
                nc.tensor.matmul(
                    psc[:], csc[:, ts(tr, 128)], csc[:],
                    start=True, stop=True,
                )
                nc.vector.tensor_mul(p_sb[:, tr, :], psc[:], tri[:, tr, :])

            # ---- (d) prefix state = (CS^T X) @ Wv  (+ msum*bv) ----
            # bb-outer so each csmN b-block is consumed as soon as it's ready
            pg = pone.tile([64, D], F32, tag="pst")
            first = True
            for bb in range(4):
                for c in range(3):
                    kt = 4 * c + bb
                    nc.tensor.matmul(
                        pg[:], csm2[:, bb, c, :, :], xn_sb[:, kt, :],
                        start=first, stop=(bb == 3 and c == 2),
                    )
                    first = False
            g_sb = cp_.tile([64, D], BF16)
            nc.vector.tensor_copy(g_sb[:], pg[:])
            gt_sb = cp_.tile([128, 4, 64], BF16)
            for kk in range(4):
                ptg = ptrp.tile([128, 128], BF16, tag="ptr")
                nc.tensor.transpose(
                    ptg[0:128, 0:64], g_sb[:, ts(kk, 128)], identb[0:64, 0:64]
                )
                nc.vector.tensor_copy(gt_sb[:, kk, :], ptg[:, 0:64])
            pst = pone.tile([64, D], F32, tag="pst")
            for kk in range(4):
                nc.tensor.matmul(
                    pst[:], gt_sb[:, kk, :], wv_sb[:, kk, :],
                    start=(kk == 0), stop=(zero_bv and kk == 3),
                )
            if not zero_bv:
                # msum[j] = sum_l CS[l, j]; state += msum (x) bv
                pms = ptrp.tile([64, 1], F32, tag="ptr")
                first = True
                for bb in range(4):
                    for c in range(3):
                        nc.tensor.matmul(
                            pms[:], csm2[:, bb, c, :, :], onesc[:],
                            start=first, stop=(bb == 3 and c == 2),
                        )
                        first = False
                ms_sb = cp_.tile([64, 1], BF16)
                nc.vector.tensor_copy(ms_sb[:], pms[:])
                msT = cp_.tile([1, 64], BF16)
                ptm = ptrp.tile([128, 128], BF16, tag="ptr")
                nc.tensor.transpose(
                    ptm[0:1, 0:64], ms_sb[:], identb[0:64, 0:64]
                )
                nc.vector.tensor_copy(msT[:], ptm[0:1, 0:64])
                nc.tensor.matmul(pst[:], msT[:], bvr_sb[:], start=False, stop=True)
            state_sb = cp_.tile([64, D], BF16)
            nc.vector.tensor_copy(state_sb[:], pst[:])

            # ---- (f) retrieved^T [D, CH], stats interleaved ----
            retrT = cp_.tile([128, 4, CH], BF16)
            sq_sb = cp_.tile([128, 4, CH], BF16)
            ps_stat = pone.tile([1, 2 * CH], F32, tag="pst")

            def emit_retr(dd):
                pr = pmm.tile([128, CH], F32, tag="pmm")
                for tr in range(4):
                    nc.tensor.matmul(
                        pr[:], value_sb[:, tr, ts(dd, 128)], p_sb[:, tr, :],
                        start=(tr == 0), stop=False,
                    )
                nc.tensor.matmul(
                    pr[:], state_sb[:, ts(dd, 128)], csc[:],
                    start=False, stop=True,
                )
                nc.vector.tensor_copy(retrT[:, dd, :], pr[:])
                nc.vector.tensor_mul(
                    sq_sb[:, dd, :], retrT[:, dd, :], retrT[:, dd, :]
                )

            def emit_stat(dd):
                nc.tensor.matmul(
                    ps_stat[0:1, 0:CH], onesc[:], retrT[:, dd, :],
                    start=(dd == 0), stop=(dd == 3),
                )
                nc.tensor.matmul(
                    ps_stat[0:1, CH : 2 * CH], onesc[:], sq_sb[:, dd, :],
                    start=(dd == 0), stop=(dd == 3),
                )

            emit_retr(0)
            emit_retr(1)
            emit_stat(0)
            emit_retr(2)
            emit_stat(1)
            emit_retr(3)
            emit_stat(2)
            emit_stat(3)

            # ---- LayerNorm rstd in row space; DMA bounce transposes it ----
            # eps scales by norm^2 (host table) since retr is un-normalized
            mu_n = cp_.tile([1, CH], BF16)
            nc.vector.tensor_scalar_mul(mu_n[:], ps_stat[0:1, 0:CH], 1.0 / D)
            mu_f = cp_.tile([1, CH], F32)
            nc.vector.tensor_scalar_mul(mu_f[:], ps_stat[0:1, 0:CH], 1.0 / D)
            msq = cp_.tile([1, CH], F32)
            nc.vector.tensor_scalar_mul(msq[:], ps_stat[0:1, CH : 2 * CH], 1.0 / D)
            mu2 = cp_.tile([1, CH], F32)
            nc.vector.tensor_mul(mu2[:], mu_f[:], mu_f[:])
            var_e = cp_.tile([1, CH], F32)
            nc.vector.tensor_sub(var_e[:], msq[:], mu2[:])
            nc.vector.tensor_add(var_e[:], var_e[:], epsn_sb[:])
            sd = cp_.tile([1, CH], F32)
            nc.scalar.activation(sd[:], var_e[:], AF.Sqrt)
            rstd_row = cp_.tile([1, CH], F32)
            nc.vector.reciprocal(rstd_row[:], sd[:])
            rsd_d = dram.tile([1, CH], F32)
            nc.gpsimd.dma_start(rsd_d[:], rstd_row[:])
            rstdT = cp_.tile([128, 4], F32)
            nc.gpsimd.dma_start(
                rstdT[:], rsd_d[:].rearrange("o (q p) -> (o p) q", p=128)
            )

            # ---- (h) out = rstd*(retr^T @ Wo' + mu*ncs) + res ----
            for tt in range(4):
                pho = pmm.tile([128, D], F32, tag="pmm")
                for ee in range(4):
                    nc.tensor.matmul(
                        pho[:], retrT[:, ee, ts(tt, 128)], wo_sb[:, ee, :],
                        start=(ee == 0), stop=False,
                    )
                nc.tensor.matmul(
                    pho[:], mu_n[0:1, ts(tt, 128)], ncs_sb[:],
                    start=False, stop=True,
                )
                out_t = rot4.tile([128, D], F32, tag="outt")
                nc.vector.scalar_tensor_tensor(
                    out=out_t[:], in0=pho[:], scalar=rstdT[:, tt : tt + 1],
                    in1=res_sb[:, tt, :], op0=ALU.mult, op1=ALU.add,
                )
                nc.sync.dma_start(y[ts(tt, 128), :], out_t[:])

    nc.compile()
    return nc


def _get_nc(zero_bv: bool):
    key = ("nc", zero_bv)
    if key not in _NC_CACHE:
        _NC_CACHE[key] = _build(zero_bv)
    return _NC_CACHE[key]


def _prep_inputs(inputs):
    x = np.asarray(inputs["x"], np.float32)
    W1 = np.asarray(inputs["W1"], np.float32)
    b1 = np.asarray(inputs["b1"], np.float32)
    W2 = np.asarray(inputs["W2"], np.float32)
    b2 = np.asarray(inputs["b2"], np.float32)
    pos_scale = float(np.asarray(inputs["pos_scale"]).reshape(-1)[0])
    content_scale = float(np.asarray(inputs["content_scale"]).reshape(-1)[0])
    Wv = np.asarray(inputs["Wv"], np.float32)
    bv = np.asarray(inputs["bv"], np.float32)
    ln_g = np.asarray(inputs["ln_g"], np.float32)
    ln_b = np.asarray(inputs["ln_b"], np.float32)
    Wo = np.asarray(inputs["Wo"], np.float32)
    bo = np.asarray(inputs["bo"], np.float32)

    bf16 = ml_dtypes.bfloat16
    Wop = ln_g[:, None] * Wo                       # fold ln gain
    ncs_v = -Wop.sum(axis=0, dtype=np.float64).astype(np.float32)[None, :]
    res_base = (ln_b @ Wo + bo).astype(np.float32)  # fold ln bias + out bias

    # [p, k, out]: row Din = 128k+p  (exact SBUF layout, contiguous DMA)
    w1_t = np.ascontiguousarray(W1.reshape(4, 128, D).transpose(1, 0, 2))
    w2_t = np.ascontiguousarray(W2.reshape(4, 128, K).transpose(1, 0, 2))
    wv_t = np.ascontiguousarray(Wv.reshape(4, 128, D).transpose(1, 0, 2))
    wo_t = np.ascontiguousarray(Wop.reshape(4, 128, D).transpose(1, 0, 2))
    b1p = np.ascontiguousarray(b1.reshape(4, 128).T)
    b2p = np.tile(b2, 4)[:, None].astype(np.float32)
    kc = np.full((128, 1), np.pi * content_scale, np.float32)
    bvr = bv[None, :].astype(np.float32)

    freqs = 1.0 / (10000.0 ** (np.arange(K, dtype=np.float64) / K))

    def packN(a):
        # [T, K] -> [128p, 4b, (4c 32f)]: natural row l = 512c + 128b + p
        t = a.reshape(4, 4, 128, K)  # [c, b, p, f]
        return np.ascontiguousarray(
            t.transpose(2, 1, 0, 3).reshape(128, 4, 4 * K))

    in_maps = []
    for core in range(NCORE):
        b, i = divmod(core, 4)
        pad = 1536 - 512 * i
        nreal = 512 * (i + 1)
        xpad = np.zeros((T, D), np.float32)
        xpad[pad:] = x[b, :nreal]
        # xt dram layout: [c, 128, 4, CH]: [p, k] = Din 128k+p, per-chunk contiguous
        xt = np.ascontiguousarray(
            xpad.T.reshape(4, 128, 4, CH).transpose(2, 1, 0, 3))

        lidx = np.arange(T, dtype=np.float64) - pad
        ang = pos_scale * lidx[:, None] * freqs[None, :]      # [T, K]
        # S path: sin(ang + ct) -> fold ang = ps + pi*n, ps in [-pi/2, pi/2]
        n_s = np.round(ang / np.pi)
        ps_f = (ang - np.pi * n_s).astype(np.float32)
        sg_s = np.where(n_s % 2 == 0, 1.0, -1.0).astype(np.float32)
        # C path: cos(ang + ct) = sin(pi/2 + ang + ct)
        n_c = np.round((ang + np.pi / 2) / np.pi)
        pc_f = (ang + np.pi / 2 - np.pi * n_c).astype(np.float32)
        sg_c = np.where(n_c % 2 == 0, 1.0, -1.0).astype(np.float32)
        # padded rows contribute nothing: zero the signs (C = S = 0)
        sg_s[lidx < 0] = 0.0
        sg_c[lidx < 0] = 0.0
        ps_f[lidx < 0] = 0.0
        pc_f[lidx < 0] = 0.0

        tblN_a = np.stack([packN(pc_f), packN(ps_f)], axis=1)  # [128, 2, 4, 128]
        sgnN_a = np.stack([packN(sg_c), packN(sg_s)], axis=1)
        # own chunk, freq-major [path*32+f, t]
        tblF_a = np.concatenate([pc_f[1536:].T, ps_f[1536:].T], axis=0)
        sgnF_a = np.concatenate([sg_c[1536:].T, sg_s[1536:].T], axis=0)
        epsn_a = (1e-5 * (np.arange(512 * i + 1, 512 * i + CH + 1,
                                    dtype=np.float64) * K)).astype(np.float32)[None, :]

        resc = (x[b, 512 * i : 512 * i + CH] + res_base[None, :]).astype(np.float32)

        xnat = np.ascontiguousarray(
            xpad[0:1536].reshape(12, 128, D).transpose(1, 0, 2))
        in_maps.append({
            "xt": xt.astype(bf16), "xn": xnat.astype(bf16),
            "w1f": np.ascontiguousarray(w1_t[:, :, 0:128].astype(bf16)),
            "w1": w1_t.astype(bf16), "w2": w2_t.astype(bf16),
            "wv": wv_t.astype(bf16), "wo": wo_t.astype(bf16),
            "ncs": ncs_v.astype(bf16), "bvr": bvr.astype(bf16),
            "b1p": b1p, "b2p": b2p, "kc": kc,
            "tblN": np.ascontiguousarray(tblN_a.astype(np.float32)),
            "sgnN": np.ascontiguousarray(sgnN_a.astype(bf16)),
            "tblF": np.ascontiguousarray(tblF_a.astype(np.float32)),
            "sgnF": np.ascontiguousarray(sgnF_a.astype(bf16)),
            "epsn": epsn_a,
            "res": np.ascontiguousarray(resc.reshape(4, 128, D).transpose(1, 0, 2)),
        })
    return in_maps


def kernel(**inputs) -> np.ndarray:
    global LAST_RESULT
    zero_bv = bool(np.all(np.asarray(inputs["bv"]) == 0.0))
    nc = _get_nc(zero_bv)
    in_maps = _prep_inputs(inputs)
    result = run_bass_kernel_spmd(
        nc, in_maps, core_ids=list(range(NCORE)), **RUN_KWARGS
    )
    LAST_RESULT = result
    y = np.empty((B, L, D), np.float32)
    for core in range(NCORE):
        b, i = divmod(core, 4)
        y[b, 512 * i : 512 * i + CH] = result.results[core]["y"]
    return y


# revision 11
# speedup vs baseline: 1.2812x; 1.0620x over previous
"""Trainium2 Bass kernel for nn_DualAddressingPhasor.

Math: the phasor cumsum-bind/retrieve is causal linear attention:
  retrieved[l] = sum_{l'<=l} (sum_k cos(phi_l,k - phi_l',k)) * value[l']
Per 512-row chunk this is (1) a carried state [2K, D] = CS^T @ value over
the prefix plus (2) intra-chunk attention triu(Cc@Cc^T + Sc@Sc^T) @ value_c.

Sharding: 8 cores = 2 batches x 4 sequence chunks of 512. Uniform SPMD
program; per-core variation is entirely in the data (right-aligned
zero-padded prefix, host-precomputed positional phase tables with zero
signs in the padding so padded rows contribute nothing).

v2: all-bf16 matmul operands (fp32r matmuls pay a serial ~70ns weight
load per matmul; bf16 hits the 216ns/512-row peak), natural-layout
phases derived by transposing the content tile tt (4 transposes instead
of 8), LayerNorm rstd computed in row space with a DMA round-trip
through DRAM to transpose [1,512]->[128,4] (replaces 8 PE transposes),
stats matmuls interleaved with the retrieve matmuls, and a DMA plan
that spreads x across queues so the PE starts early and never stalls.
"""

import sys

for _p in ("/opt/trn_rl_repo",):
    if _p not in sys.path:
        sys.path.append(_p)

import numpy as np
import ml_dtypes

import concourse.bacc as bacc
import concourse.tile as tile
import concourse.mybir as mybir
from concourse.bass import ts
from concourse.bass_utils import run_bass_kernel_spmd
from concourse.masks import make_identity

F32 = mybir.dt.float32
BF16 = mybir.dt.bfloat16
AF = mybir.ActivationFunctionType
ALU = mybir.AluOpType

D = 512
K = 32
B = 2
L = 2048
CH = 512          # chunk rows per core
T = 2048          # padded rows processed per core
NCORE = 8

_NC_CACHE = {}
LAST_RESULT = None
RUN_KWARGS = {}


def _build(zero_bv: bool):
    nc = bacc.Bacc("TRN2", num_devices=NCORE)

    xt = nc.dram_tensor("xt", [4, 128, 4, CH], BF16, kind="ExternalInput")
    w1f = nc.dram_tensor("w1f", [128, 1, D], BF16, kind="ExternalInput")
    w1 = nc.dram_tensor("w1", [128, 4, D], BF16, kind="ExternalInput")
    w2 = nc.dram_tensor("w2", [128, 4, K], BF16, kind="ExternalInput")
    wv = nc.dram_tensor("wv", [128, 4, D], BF16, kind="ExternalInput")
    wo = nc.dram_tensor("wo", [128, 4, D], BF16, kind="ExternalInput")
    ncs = nc.dram_tensor("ncs", [1, D], BF16, kind="ExternalInput")
    bvr = nc.dram_tensor("bvr", [1, D], BF16, kind="ExternalInput")
    b1p = nc.dram_tensor("b1p", [128, 4], F32, kind="ExternalInput")
    b2p = nc.dram_tensor("b2p", [128, 1], F32, kind="ExternalInput")
    kc = nc.dram_tensor("kc", [128, 1], F32, kind="ExternalInput")
    tblN = nc.dram_tensor("tblN", [128, 2, 4, 128], F32, kind="ExternalInput")
    sgnN = nc.dram_tensor("sgnN", [128, 2, 4, 128], BF16, kind="ExternalInput")
    tblF = nc.dram_tensor("tblF", [64, CH], F32, kind="ExternalInput")
    sgnF = nc.dram_tensor("sgnF", [64, CH], BF16, kind="ExternalInput")
    epsn = nc.dram_tensor("epsn", [128, 4], F32, kind="ExternalInput")
    res = nc.dram_tensor("res", [128, 4, D], F32, kind="ExternalInput")
    xn = nc.dram_tensor("xn", [128, 12, D], BF16, kind="ExternalInput")
    y = nc.dram_tensor("y", [CH, D], F32, kind="ExternalOutput")

    with tile.TileContext(nc) as tc:
        with (
            tc.tile_pool(name="const", bufs=1) as cp_,
            tc.tile_pool(name="big", bufs=1) as bigp,
            tc.tile_pool(name="rot", bufs=3) as rot,
            tc.tile_pool(name="rot4", bufs=4) as rot4,
            tc.tile_pool(name="pmm", bufs=4, space="PSUM") as pmm,
            tc.tile_pool(name="pone", bufs=1, space="PSUM") as pone,
            tc.tile_pool(name="ptr", bufs=2, space="PSUM") as ptrp,
            tc.tile_pool(name="dram", bufs=1, space="DRAM") as dram,
        ):
            # ---- input loads, spread over queues so (a) starts ASAP ----
            xt_sb = bigp.tile([128, 4, T], BF16)
            w1_sb = cp_.tile([128, 4, D], BF16)
            w2_sb = cp_.tile([128, 4, K], BF16)
            wv_sb = cp_.tile([128, 4, D], BF16)
            wo_sb = cp_.tile([128, 4, D], BF16)
            res_sb = cp_.tile([128, 4, D], F32)
            bvr_sb = cp_.tile([1, D], BF16)
            ncs_sb = cp_.tile([1, D], BF16)
            b1p_sb = cp_.tile([128, 4], F32)
            b2p_sb = cp_.tile([128, 1], F32)
            kc_sb = cp_.tile([128, 1], F32)
            tblN_sb = cp_.tile([128, 2, 4, 128], F32)
            sgnN_sb = cp_.tile([128, 2, 4, 128], BF16)
            tblF_sb = cp_.tile([64, CH], F32)
            sgnF_sb = cp_.tile([64, CH], BF16)
            epsn_sb = cp_.tile([128, 4], F32)
            xn_sb = bigp.tile([128, 12, D], BF16)

            # sync: first-need pair then mid-kernel needs
            nc.sync.dma_start(w1_sb[:, 0:1, :], w1f[:])
            nc.sync.dma_start(xt_sb[:, 0, ts(0, CH)], xt[0][:, 0, :])
            nc.sync.dma_start(xt_sb[:, 1:4, ts(0, CH)], xt[0][:, 1:4, :])
            nc.sync.dma_start(xt_sb[:, :, ts(2, CH)], xt[2])
            nc.sync.dma_start(xn_sb[:], xn[:])
            nc.sync.dma_start(tblN_sb[:], tblN[:])
            nc.sync.dma_start(sgnN_sb[:], sgnN[:])
            nc.sync.dma_start(ncs_sb[:], ncs[:])
            nc.sync.dma_start(bvr_sb[:], bvr[:])
            nc.sync.dma_start(epsn_sb[:], epsn[:])
            # gpsimd: w1 rest, then chunks 1/3, then late-need
            nc.gpsimd.dma_start(w1_sb[:, 1:4, :], w1[:, 1:4, :])
            nc.gpsimd.dma_start(xt_sb[:, :, ts(1, CH)], xt[1])
            nc.gpsimd.dma_start(xt_sb[:, :, ts(3, CH)], xt[3])
            nc.gpsimd.dma_start(w2_sb[:], w2[:])
            nc.gpsimd.dma_start(wv_sb[:], wv[:])
            nc.gpsimd.dma_start(tblF_sb[:], tblF[:])
            nc.gpsimd.dma_start(sgnF_sb[:], sgnF[:])
            nc.gpsimd.dma_start(res_sb[:], res[:])
            nc.gpsimd.dma_start(wo_sb[:], wo[:])
            # scalar: only the small early tables (keeps the act-table
            # loads near the queue head so the first tanh isn't delayed)
            nc.scalar.dma_start(b1p_sb[:], b1p[:])
            nc.scalar.dma_start(kc_sb[:], kc[:])
            nc.scalar.dma_start(b2p_sb[:], b2p[:])

            onesf = cp_.tile([128, 128], F32)
            nc.vector.memset(onesf[:], 1.0)
            # preload the Sin/Sqrt activation tables so their 1.5us loads
            # don't land on the critical path mid-kernel
            dums = cp_.tile([1, 8], F32)
            nc.scalar.activation(dums[:], onesf[0:1, 0:8], AF.Sin)
            nc.scalar.activation(dums[:], onesf[0:1, 0:8], AF.Sqrt)
            onesr = cp_.tile([1, 128], BF16)
            nc.vector.tensor_copy(onesr[:], onesf[0:1, :])
            onesc = cp_.tile([128, 1], BF16)
            nc.vector.tensor_copy(onesc[:], onesf[:, 0:1])

            identb = cp_.tile([128, 128], BF16)
            make_identity(nc, identb[:])

            # triangular masks for intra-chunk causal attention (lhsT form:
            # tri[p, tr, y] = 1 iff y >= p + 128*tr)
            tri = cp_.tile([128, 4, CH], BF16)
            for tr in range(4):
                nc.gpsimd.memset(tri[:, tr, :], 0.0)
                nc.gpsimd.affine_select(
                    out=tri[:, tr, :], in_=tri[:, tr, :],
                    compare_op=ALU.is_gt, fill=1.0, base=128 * tr,
                    pattern=[[-1, CH]], channel_multiplier=1,
                )

            # ---- (a) h^T = tanh(W1^T x^T + b1) per chunk; (b) content tt,
            # deferred one chunk so the tanh latency hides under (a) ----
            tt_sb = cp_.tile([128, CH], BF16)
            h_cks = [None] * 4

            def emit_b(c):
                pc = pmm.tile([32, CH], F32, tag="pmm")
                for k in range(4):
                    nc.tensor.matmul(
                        pc[:], w2_sb[:, k, :], h_cks[c][:, k, :],
                        start=(k == 0), stop=(k == 3),
                    )
                nc.scalar.activation(
                    tt_sb[32 * c : 32 * c + 32, :], pc[:], AF.Tanh,
                    bias=b2p_sb[0:32, :], scale=1.0,
                )

            for c in range(4):
                h_ck = rot.tile([128, 4, CH], BF16, tag="hck")
                h_cks[c] = h_ck
                if c == 0:
                    # k-outer: consume the k-split chunk-0 DMAs as they land
                    phs = [pmm.tile([128, CH], F32, tag="pmm", name=f"ph0_{d}") for d in range(4)]
                    for k in range(4):
                        for dout in range(4):
                            nc.tensor.matmul(
                                phs[dout][:], w1_sb[:, k, ts(dout, 128)],
                                xt_sb[:, k, ts(0, CH)],
                                start=(k == 0), stop=(k == 3),
                            )
                    for dout in range(4):
                        nc.scalar.activation(
                            h_ck[:, dout, :], phs[dout][:], AF.Tanh,
                            bias=b1p_sb[:, dout : dout + 1], scale=1.0,
                        )
                else:
                    for dout in range(4):
                        ph = pmm.tile([128, CH], F32, tag="pmm")
                        for k in range(4):
                            nc.tensor.matmul(
                                ph[:], w1_sb[:, k, ts(dout, 128)],
                                xt_sb[:, k, ts(c, CH)],
                                start=(k == 0), stop=(k == 3),
                            )
                        nc.scalar.activation(
                            h_ck[:, dout, :], ph[:], AF.Tanh,
                            bias=b1p_sb[:, dout : dout + 1], scale=1.0,
                        )
                if c >= 1:
                    emit_b(c - 1)
            emit_b(3)

            # ---- freq-major phases for the own chunk (csc [64, CH]) ----
            # S/C = sgn * sin(tbl + kc*tt); host folds the positional part to
            # [-pi/2, pi/2] plus a sign so the Sin LUT stays accurate.
            ttF = cp_.tile([64, CH], BF16)
            nc.vector.tensor_copy(ttF[0:32, :], tt_sb[96:128, :])
            nc.vector.tensor_copy(ttF[32:64, :], tt_sb[96:128, :])
            argF = cp_.tile([64, CH], F32)
            nc.vector.scalar_tensor_tensor(
                out=argF[:], in0=ttF[:], scalar=kc_sb[0:64, :], in1=tblF_sb[:],
                op0=ALU.mult, op1=ALU.add,
            )
            sinF = cp_.tile([64, CH], F32)
            nc.scalar.activation(sinF[:], argF[:], AF.Sin)
            csc = cp_.tile([64, CH], BF16)
            nc.vector.tensor_mul(csc[:], sinF[:], sgnF_sb[:])

            # ---- (c) value = x @ Wv (+bv) for the own chunk ----
            value_sb = bigp.tile([128, 4, D], BF16)

            def emit_value(tt):
                pv = pmm.tile([128, D], F32, tag="pmm")
                for k in range(4):
                    nc.tensor.matmul(
                        pv[:], xt_sb[:, k, ts(12 + tt, 128)], wv_sb[:, k, :],
                        start=(k == 0), stop=(zero_bv and k == 3),
                    )
                if not zero_bv:
                    nc.tensor.matmul(pv[:], onesr[:], bvr_sb[:], start=False, stop=True)
                if tt % 2 == 0:
                    nc.vector.tensor_copy(value_sb[:, tt, :], pv[:])
                else:
                    nc.scalar.copy(value_sb[:, tt, :], pv[:])

            emit_value(0)
            emit_value(1)

            # ---- natural-layout phases: transpose tt, then sin per b-block ----
            # ttN[p, b, 32c+f] = tt[32c+f, 128b+p]; natural row l = 512c+128b+p
            ttN = cp_.tile([128, 4, 128], BF16)
            for b in range(4):
                ptr_ = ptrp.tile([128, 128], BF16, tag="ptr")
                nc.tensor.transpose(ptr_[:], tt_sb[:, ts(b, 128)], identb[:])
                nc.vector.tensor_copy(ttN[:, b, :], ptr_[:])

            emit_value(2)
            emit_value(3)

            argN = cp_.tile([128, 2, 4, 128], F32)
            for path in range(2):
                nc.vector.scalar_tensor_tensor(
                    out=argN[:, path], in0=ttN[:], scalar=kc_sb[:],
                    in1=tblN_sb[:, path], op0=ALU.mult, op1=ALU.add,
                )
            # csm2[p, b, c, path, f]: (path, f) contiguous so the pg lhsT
            # slice coalesces to a 2D [128, 64] access pattern
            csm2 = cp_.tile([128, 4, 4, 2, 32], BF16)
            sinN = cp_.tile([128, 2, 4, 128], F32)
            for b in range(4):
                nc.scalar.activation(sinN[:, :, b, :], argN[:, :, b, :], AF.Sin)
                for path in range(2):
                    eng = nc.vector if path == 0 else nc.gpsimd
                    eng.tensor_mul(
                        csm2[:, b, :, path, :],
                        sinN[:, path, b, :].rearrange("p (c f) -> p c f", f=32),
                        sgnN_sb[:, path, b, :].rearrange("p (c f) -> p c f", f=32),
                    )

            # ---- (e) intra-chunk scores, triu-masked ----
            p_sb = cp_.tile([128, 4, CH], BF16)
            for tr in range(4):
                psc = pmm.tile([128, CH], F32, tag="pmm")
                nc.tensor.matmul(
                    psc[:], csc[:, ts(tr, 128)], csc[:],
                    start=True, stop=True,
                )
                nc.vector.tensor_mul(p_sb[:, tr, :], psc[:], tri[:, tr, :])

            # ---- (d) prefix state = (CS^T X) @ Wv  (+ msum*bv) ----
            # bb-outer so each csmN b-block is consumed as soon as it's ready
            pg = pone.tile([64, D], F32, tag="pst")
            first = True
            for bb in range(4):
                for c in range(3):
                    kt = 4 * c + bb
                    nc.tensor.matmul(
                        pg[:], csm2[:, bb, c, :, :], xn_sb[:, kt, :],
                        start=first, stop=(bb == 3 and c == 2),
                    )
                    first = False
            g_sb = cp_.tile([64, D], BF16)
            nc.vector.tensor_copy(g_sb[:], pg[:])
            gt_sb = cp_.tile([128, 4, 64], BF16)
            for kk in range(4):
                ptg = ptrp.tile([128, 128], BF16, tag="ptr")
                nc.tensor.transpose(
                    ptg[0:128, 0:64], g_sb[:, ts(kk, 128)], identb[0:64, 0:64]
                )
                nc.vector.tensor_copy(gt_sb[:, kk, :], ptg[:, 0:64])
            pst = pone.tile([64, D], F32, tag="pst")
            for kk in range(4):
                nc.tensor.matmul(
                    pst[:], gt_sb[:, kk, :], wv_sb[:, kk, :],
                    start=(kk == 0), stop=(zero_bv and kk == 3),
                )
            if not zero_bv:
                # msum[j] = sum_l CS[l, j]; state += msum (x) bv
                pms = ptrp.tile([64, 1], F32, tag="ptr")
                first = True
                for bb in range(4):
                    for c in range(3):
                        nc.tensor.matmul(
                            pms[:], csm2[:, bb, c, :, :], onesc[:],
                            start=first, stop=(bb == 3 and c == 2),
                        )
                        first = False
                ms_sb = cp_.tile([64, 1], BF16)
                nc.vector.tensor_copy(ms_sb[:], pms[:])
                msT = cp_.tile([1, 64], BF16)
                ptm = ptrp.tile([128, 128], BF16, tag="ptr")
                nc.tensor.transpose(
                    ptm[0:1, 0:64], ms_sb[:], identb[0:64, 0:64]
                )
                nc.vector.tensor_copy(msT[:], ptm[0:1, 0:64])
                nc.tensor.matmul(pst[:], msT[:], bvr_sb[:], start=False, stop=True)
            state_sb = cp_.tile([64, D], BF16)
            nc.vector.tensor_copy(state_sb[:], pst[:])

            # ---- (f) retrieved^T [D, CH], stats interleaved ----
            retrT = cp_.tile([128, 4, CH], BF16)
            sq_sb = cp_.tile([128, 4, CH], BF16)
            ps_stat = pone.tile([1, 2 * CH], F32, tag="pst")

            def emit_retr(dd):
                pr = pmm.tile([128, CH], F32, tag="pmm")
                for tr in range(4):
                    nc.tensor.matmul(
                        pr[:], value_sb[:, tr, ts(dd, 128)], p_sb[:, tr, :],
                        start=(tr == 0), stop=False,
                    )
                nc.tensor.matmul(
                    pr[:], state_sb[:, ts(dd, 128)], csc[:],
                    start=False, stop=True,
                )
                if dd % 2 == 0:
                    nc.scalar.copy(retrT[:, dd, :], pr[:])
                else:
                    nc.vector.tensor_copy(retrT[:, dd, :], pr[:])
                nc.gpsimd.tensor_mul(
                    sq_sb[:, dd, :], retrT[:, dd, :], retrT[:, dd, :]
                )

            def emit_stat(dd):
                nc.tensor.matmul(
                    ps_stat[0:1, 0:CH], onesc[:], retrT[:, dd, :],
                    start=(dd == 0), stop=(dd == 3),
                )
                nc.tensor.matmul(
                    ps_stat[0:1, CH : 2 * CH], onesc[:], sq_sb[:, dd, :],
                    start=(dd == 0), stop=(dd == 3),
                )

            emit_retr(0)
            emit_retr(1)
            emit_stat(0)
            emit_retr(2)
            emit_stat(1)
            emit_retr(3)
            emit_stat(2)
            emit_stat(3)

            # ---- LayerNorm rstd: bounce raw sums through DRAM into a
            # column layout [128, 8], then tiny per-partition math ----
            mu_n = cp_.tile([1, CH], BF16)
            nc.vector.tensor_scalar_mul(mu_n[:], ps_stat[0:1, 0:CH], 1.0 / D)
            stat_row = cp_.tile([1, 2 * CH], F32)
            nc.vector.tensor_copy(stat_row[0:1, 0:CH], ps_stat[0:1, 0:CH])
            nc.scalar.copy(stat_row[0:1, CH:], ps_stat[0:1, CH:])
            stat_d = dram.tile([1, 2 * CH], F32)
            nc.gpsimd.dma_start(stat_d[:], stat_row[:])
            statsT = cp_.tile([128, 8], F32)
            nc.gpsimd.dma_start(
                statsT[:], stat_d[:].rearrange("o (q p) -> (o p) q", p=128)
            )
            muT = cp_.tile([128, 4], F32)
            nc.vector.tensor_scalar_mul(muT[:], statsT[:, 0:4], 1.0 / D)
            varT = cp_.tile([128, 4], F32)
            nc.vector.tensor_scalar_mul(varT[:], statsT[:, 4:8], 1.0 / D)
            mu2T = cp_.tile([128, 4], F32)
            nc.vector.tensor_mul(mu2T[:], muT[:], muT[:])
            nc.vector.tensor_sub(varT[:], varT[:], mu2T[:])
            nc.vector.tensor_add(varT[:], varT[:], epsn_sb[:])
            sdT = cp_.tile([128, 4], F32)
            nc.scalar.activation(sdT[:], varT[:], AF.Sqrt)
            rstdT = cp_.tile([128, 4], F32)
            nc.vector.reciprocal(rstdT[:], sdT[:])

            # ---- (h) out = rstd*(retr^T @ Wo' + mu*ncs) + res ----
            for tt in range(4):
                pho = pmm.tile([128, D], F32, tag="pmm")
                for ee in range(4):
                    nc.tensor.matmul(
                        pho[:], retrT[:, ee, ts(tt, 128)], wo_sb[:, ee, :],
                        start=(ee == 0), stop=False,
                    )
                nc.tensor.matmul(
                    pho[:], mu_n[0:1, ts(tt, 128)], ncs_sb[:],
                    start=False, stop=True,
                )
                out_t = rot4.tile([128, D], F32, tag="outt")
                if tt % 2 == 0:
                    nc.vector.scalar_tensor_tensor(
                        out=out_t[:], in0=pho[:], scalar=rstdT[:, tt : tt + 1],
                        in1=res_sb[:, tt, :], op0=ALU.mult, op1=ALU.add,
                    )
                else:
                    tmp_t = rot4.tile([128, D], F32, tag="tmpt")
                    nc.scalar.mul(tmp_t[:], pho[:], rstdT[:, tt : tt + 1])
                    nc.gpsimd.tensor_add(out_t[:], tmp_t[:], res_sb[:, tt, :])
                deng = nc.sync if tt % 2 == 0 else nc.scalar
                deng.dma_start(y[ts(tt, 128), :], out_t[:])

    nc.compile()
    return nc


def _get_nc(zero_bv: bool):
    key = ("nc", zero_bv)
    if key not in _NC_CACHE:
        _NC_CACHE[key] = _build(zero_bv)
    return _NC_CACHE[key]


def _prep_inputs(inputs):
    x = np.asarray(inputs["x"], np.float32)
    W1 = np.asarray(inputs["W1"], np.float32)
    b1 = np.asarray(inputs["b1"], np.float32)
    W2 = np.asarray(inputs["W2"], np.float32)
    b2 = np.asarray(inputs["b2"], np.float32)
    pos_scale = float(np.asarray(inputs["pos_scale"]).reshape(-1)[0])
    content_scale = float(np.asarray(inputs["content_scale"]).reshape(-1)[0])
    Wv = np.asarray(inputs["Wv"], np.float32)
    bv = np.asarray(inputs["bv"], np.float32)
    ln_g = np.asarray(inputs["ln_g"], np.float32)
    ln_b = np.asarray(inputs["ln_b"], np.float32)
    Wo = np.asarray(inputs["Wo"], np.float32)
    bo = np.asarray(inputs["bo"], np.float32)

    bf16 = ml_dtypes.bfloat16
    Wop = ln_g[:, None] * Wo                       # fold ln gain
    ncs_v = -Wop.sum(axis=0, dtype=np.float64).astype(np.float32)[None, :]
    res_base = (ln_b @ Wo + bo).astype(np.float32)  # fold ln bias + out bias

    # [p, k, out]: row Din = 128k+p  (exact SBUF layout, contiguous DMA)
    w1_t = np.ascontiguousarray(W1.reshape(4, 128, D).transpose(1, 0, 2))
    w2_t = np.ascontiguousarray(W2.reshape(4, 128, K).transpose(1, 0, 2))
    wv_t = np.ascontiguousarray(Wv.reshape(4, 128, D).transpose(1, 0, 2))
    wo_t = np.ascontiguousarray(Wop.reshape(4, 128, D).transpose(1, 0, 2))
    b1p = np.ascontiguousarray(b1.reshape(4, 128).T)
    b2p = np.tile(b2, 4)[:, None].astype(np.float32)
    kc = np.full((128, 1), np.pi * content_scale, np.float32)
    bvr = bv[None, :].astype(np.float32)

    freqs = 1.0 / (10000.0 ** (np.arange(K, dtype=np.float64) / K))

    def packN(a):
        # [T, K] -> [128p, 4b, (4c 32f)]: natural row l = 512c + 128b + p
        t = a.reshape(4, 4, 128, K)  # [c, b, p, f]
        return np.ascontiguousarray(
            t.transpose(2, 1, 0, 3).reshape(128, 4, 4 * K))

    in_maps = []
    for core in range(NCORE):
        b, i = divmod(core, 4)
        pad = 1536 - 512 * i
        nreal = 512 * (i + 1)
        xpad = np.zeros((T, D), np.float32)
        xpad[pad:] = x[b, :nreal]
        # xt dram layout: [c, 128, 4, CH]: [p, k] = Din 128k+p, per-chunk contiguous
        xt = np.ascontiguousarray(
            xpad.T.reshape(4, 128, 4, CH).transpose(2, 1, 0, 3))

        lidx = np.arange(T, dtype=np.float64) - pad
        ang = pos_scale * lidx[:, None] * freqs[None, :]      # [T, K]
        # S path: sin(ang + ct) -> fold ang = ps + pi*n, ps in [-pi/2, pi/2]
        n_s = np.round(ang / np.pi)
        ps_f = (ang - np.pi * n_s).astype(np.float32)
        sg_s = np.where(n_s % 2 == 0, 1.0, -1.0).astype(np.float32)
        # C path: cos(ang + ct) = sin(pi/2 + ang + ct)
        n_c = np.round((ang + np.pi / 2) / np.pi)
        pc_f = (ang + np.pi / 2 - np.pi * n_c).astype(np.float32)
        sg_c = np.where(n_c % 2 == 0, 1.0, -1.0).astype(np.float32)
        # padded rows contribute nothing: zero the signs (C = S = 0)
        sg_s[lidx < 0] = 0.0
        sg_c[lidx < 0] = 0.0
        ps_f[lidx < 0] = 0.0
        pc_f[lidx < 0] = 0.0

        tblN_a = np.stack([packN(pc_f), packN(ps_f)], axis=1)  # [128, 2, 4, 128]
        sgnN_a = np.stack([packN(sg_c), packN(sg_s)], axis=1)
        # own chunk, freq-major [path*32+f, t]
        tblF_a = np.concatenate([pc_f[1536:].T, ps_f[1536:].T], axis=0)
        sgnF_a = np.concatenate([sg_c[1536:].T, sg_s[1536:].T], axis=0)
        epsn_r = (1e-5 * (np.arange(512 * i + 1, 512 * i + CH + 1,
                                    dtype=np.float64) * K)).astype(np.float32)
        epsn_a = np.ascontiguousarray(epsn_r.reshape(4, 128).T)  # [128p, 4tt]

        resc = (x[b, 512 * i : 512 * i + CH] + res_base[None, :]).astype(np.float32)

        xnat = np.ascontiguousarray(
            xpad[0:1536].reshape(12, 128, D).transpose(1, 0, 2))
        in_maps.append({
            "xt": xt.astype(bf16), "xn": xnat.astype(bf16),
            "w1f": np.ascontiguousarray(w1_t[:, 0:1, :].astype(bf16)),
            "w1": w1_t.astype(bf16), "w2": w2_t.astype(bf16),
            "wv": wv_t.astype(bf16), "wo": wo_t.astype(bf16),
            "ncs": ncs_v.astype(bf16), "bvr": bvr.astype(bf16),
            "b1p": b1p, "b2p": b2p, "kc": kc,
            "tblN": np.ascontiguousarray(tblN_a.astype(np.float32)),
            "sgnN": np.ascontiguousarray(sgnN_a.astype(bf16)),
            "tblF": np.ascontiguousarray(tblF_a.astype(np.float32)),
            "sgnF": np.ascontiguousarray(sgnF_a.astype(bf16)),
            "epsn": epsn_a,
            "res": np.ascontiguousarray(resc.reshape(4, 128, D).transpose(1, 0, 2)),
        })
    return in_maps


def kernel(**inputs) -> np.ndarray:
    global LAST_RESULT
    zero_bv = bool(np.all(np.asarray(inputs["bv"]) == 0.0))
    nc = _get_nc(zero_bv)
    in_maps = _prep_inputs(inputs)
    result = run_bass_kernel_spmd(
        nc, in_maps, core_ids=list(range(NCORE)), **RUN_KWARGS
    )
    LAST_RESULT = result
    y = np.empty((B, L, D), np.float32)
    for core in range(NCORE):
        b, i = divmod(core, 4)
        y[b, 512 * i : 512 * i + CH] = result.results[core]["y"]
    return y
